# revision 6
# baseline (speedup 1.0000x reference)
"""Trainium2 Bass kernel for nn_Contextual_MFN (Memory Fusion Network), v2.

Fully fused chunk pipeline (8-step chunks), SBUF/PSUM resident, no DRAM
spills.  Per step-slot the issue order interleaves:
  p1 (LSTM, chunk k)  |  p2a (attention, chunk k-1)  |  p3 (memory, chunk k-2)
with phase-0 (Wih@x -> gate PSUM) running one 4-step group ahead.

Tricks:
  - gates accumulate in PSUM: p0 f32r matmuls write xWb, p1 bf16 Whh matmuls
    accumulate start=False, ACT reads PSUM directly (no copy, no inject).
  - no sigmoid anywhere: carry C=2c, M=2mem; sigmoid(x) = 0.5*tanh(x/2)+0.5
    folds into scalar_tensor_tensor ops + host-side 0.5 weight scalings.
    Whole kernel uses {tanh, relu, exp, identity} = one ACT table set.
  - bf16 for all small serial matmuls (no fp32 LOW/HIGH split, FWL weight
    loads); attention path f32r at N=256 (full rate).
"""
import numpy as np
import ml_dtypes

import concourse.bass as bass
import concourse.tile as tile
from concourse import bacc, mybir
from concourse.bass_utils import run_bass_kernel_spmd

F32 = mybir.dt.float32
F32R = mybir.dt.float32r
BF16 = mybir.dt.bfloat16
AF = mybir.ActivationFunctionType
ALU = mybir.AluOpType

T_FULL = 512
NBATCH = 256
NCORES = 8
B = NBATCH // NCORES          # 32 batch rows per core
D_L, D_A, D_V = 300, 74, 35
DIN = D_L + D_A + D_V         # 409
DAUG = DIN + 1                # 410 (ones row for bias)
DH = 128
MEM = 256
CH = 8                        # chunk (steps)
GRP = 4                       # phase-0 gate group (steps)

# gate slot order: s = g'*3 + m, g' in (i, f, o, g_tanh); torch rows (i,f,g,o)
TORCH_G = (0, 1, 3, 2)


def _nonzero_kcs(s):
    m = s % 3
    if m == 0:
        return [0, 1, 2, 3]
    return [2, 3]


def build_program(Tp=T_FULL):
    assert Tp % CH == 0
    NCH = Tp // CH
    NGRP = Tp // GRP
    nc = bacc.Bacc("TRN2", target_bir_lowering=False, debug=False)

    # ---------------- external inputs ----------------
    xT = nc.dram_tensor("xT", [DAUG, Tp * B], F32, kind="ExternalInput")
    waug = nc.dram_tensor("waug", [512, 1536], BF16, kind="ExternalInput")
    whhT = nc.dram_tensor("whhT", [128, 1536], BF16, kind="ExternalInput")
    ones128 = nc.dram_tensor("ones128", [128, 1], BF16, kind="ExternalInput")

    a1w1 = nc.dram_tensor("a1w1", [768, 256], BF16, kind="ExternalInput")
    a1b1 = nc.dram_tensor("a1b1", [128, 2], F32, kind="ExternalInput")
    a1w2 = nc.dram_tensor("a1w2", [256, 768], BF16, kind="ExternalInput")
    a1b2 = nc.dram_tensor("a1b2", [128, 6], F32, kind="ExternalInput")
    a2w1 = nc.dram_tensor("a2w1", [768, 256], BF16, kind="ExternalInput")
    a2b1r = nc.dram_tensor("a2b1r", [1, 256], BF16, kind="ExternalInput")
    a2w2 = nc.dram_tensor("a2w2", [256, 256], BF16, kind="ExternalInput")
    a2b2r = nc.dram_tensor("a2b2r", [1, 256], BF16, kind="ExternalInput")
    g1a = nc.dram_tensor("g1a", [768, 256], BF16, kind="ExternalInput")
    g2a = nc.dram_tensor("g2a", [768, 256], BF16, kind="ExternalInput")
    g1b1r = nc.dram_tensor("g1b1r", [1, 256], BF16, kind="ExternalInput")
    g2b1r = nc.dram_tensor("g2b1r", [1, 256], BF16, kind="ExternalInput")
    g1b = nc.dram_tensor("g1b", [256, 256], BF16, kind="ExternalInput")
    g2b = nc.dram_tensor("g2b", [256, 256], BF16, kind="ExternalInput")
    g1w2 = nc.dram_tensor("g1w2", [256, 256], BF16, kind="ExternalInput")
    g2w2 = nc.dram_tensor("g2w2", [256, 256], BF16, kind="ExternalInput")
    gb2r = nc.dram_tensor("gb2r", [1, 512], BF16, kind="ExternalInput")
    ow1 = nc.dram_tensor("ow1", [640, 256], BF16, kind="ExternalInput")
    ob1 = nc.dram_tensor("ob1", [128, 2], F32, kind="ExternalInput")
    ow2 = nc.dram_tensor("ow2", [256, 1], BF16, kind="ExternalInput")
    ob2 = nc.dram_tensor("ob2", [1, 1], F32, kind="ExternalInput")

    out_d = nc.dram_tensor("out", [B, 1], F32, kind="ExternalOutput")
    dbg_h = nc.dram_tensor("dbg_h", [128, 3, 32], F32, kind="ExternalOutput")
    dbg_c = nc.dram_tensor("dbg_c", [128, 3, 32], BF16, kind="ExternalOutput")
    dbg_m = nc.dram_tensor("dbg_m", [128, 2, 32], F32, kind="ExternalOutput")
    dbg_ug = nc.dram_tensor("dbg_ug", [128, 4, 256], F32, kind="ExternalOutput")
    dbg_vt = nc.dram_tensor("dbg_vt", [128, 2, 256], F32, kind="ExternalOutput")
    dbg_c0 = nc.dram_tensor("dbg_c0", [128, 3, 9, 32], BF16, kind="ExternalOutput")
    dbg_g0 = nc.dram_tensor("dbg_g0", [128, 12, 4, 32], F32, kind="ExternalOutput")
    dbg_g1 = nc.dram_tensor("dbg_g1", [128, 12, 4, 32], F32, kind="ExternalOutput")

    import contextlib
    with tile.TileContext(nc) as tc:
        ctx = contextlib.ExitStack()
        with ctx:
            wpool = ctx.enter_context(tc.tile_pool(name="weights", bufs=1))

            # ---- resident weights / constants ----
            wihT_t = wpool.tile([128, 4, 1536], BF16)
            nc.sync.dma_start(
                wihT_t[:], waug.ap().rearrange("(kc p) c -> p kc c", p=128))
            whhT_t = wpool.tile([128, 1536], BF16)
            nc.sync.dma_start(whhT_t[:], whhT.ap())
            ones128_t = wpool.tile([128, 1], BF16)
            nc.sync.dma_start(ones128_t[:], ones128.ap())
            ones1x128_t = wpool.tile([1, 128], BF16)
            nc.sync.dma_start(ones1x128_t[:], ones128.ap().rearrange("p one -> (one) (p)"))

            a1w1_t = wpool.tile([128, 6, 256], BF16)
            nc.sync.dma_start(a1w1_t[:], a1w1.ap().rearrange("(kc p) c -> p kc c", p=128))
            a1b1_t = wpool.tile([128, 2], F32)
            nc.sync.dma_start(a1b1_t[:], a1b1.ap())
            a1w2_t = wpool.tile([128, 2, 768], BF16)
            nc.sync.dma_start(a1w2_t[:], a1w2.ap().rearrange("(kc p) c -> p kc c", p=128))
            a1b2_t = wpool.tile([128, 6], F32)
            nc.sync.dma_start(a1b2_t[:], a1b2.ap())
            a2w1_t = wpool.tile([128, 6, 256], BF16)
            nc.sync.dma_start(a2w1_t[:], a2w1.ap().rearrange("(kc p) c -> p kc c", p=128))
            a2b1r_t = wpool.tile([1, 256], BF16)
            nc.sync.dma_start(a2b1r_t[:], a2b1r.ap())
            a2w2_t = wpool.tile([128, 2, 256], BF16)
            nc.sync.dma_start(a2w2_t[:], a2w2.ap().rearrange("(kc p) c -> p kc c", p=128))
            a2b2r_t = wpool.tile([1, 256], BF16)
            nc.sync.dma_start(a2b2r_t[:], a2b2r.ap())
            g1a_t = wpool.tile([128, 6, 256], BF16)
            nc.sync.dma_start(g1a_t[:], g1a.ap().rearrange("(kc p) c -> p kc c", p=128))
            g2a_t = wpool.tile([128, 6, 256], BF16)
            nc.sync.dma_start(g2a_t[:], g2a.ap().rearrange("(kc p) c -> p kc c", p=128))
            g1b1r_t = wpool.tile([1, 256], BF16)
            nc.sync.dma_start(g1b1r_t[:], g1b1r.ap())
            g2b1r_t = wpool.tile([1, 256], BF16)
            nc.sync.dma_start(g2b1r_t[:], g2b1r.ap())
            g1b_t = wpool.tile([128, 2, 256], BF16)
            nc.sync.dma_start(g1b_t[:], g1b.ap().rearrange("(kc p) c -> p kc c", p=128))
            g2b_t = wpool.tile([128, 2, 256], BF16)
            nc.sync.dma_start(g2b_t[:], g2b.ap().rearrange("(kc p) c -> p kc c", p=128))
            g1w2_t = wpool.tile([128, 2, 256], BF16)
            nc.sync.dma_start(g1w2_t[:], g1w2.ap().rearrange("(kc p) c -> p kc c", p=128))
            g2w2_t = wpool.tile([128, 2, 256], BF16)
            nc.sync.dma_start(g2w2_t[:], g2w2.ap().rearrange("(kc p) c -> p kc c", p=128))
            gb2r_t = wpool.tile([1, 512], BF16)
            nc.sync.dma_start(gb2r_t[:], gb2r.ap())
            ow1_t = wpool.tile([128, 5, 256], BF16)
            nc.sync.dma_start(ow1_t[:], ow1.ap().rearrange("(kc p) c -> p kc c", p=128))
            ob1_t = wpool.tile([128, 2], F32)
            nc.sync.dma_start(ob1_t[:], ob1.ap())
            ow2_t = wpool.tile([128, 2, 1], BF16)
            nc.sync.dma_start(ow2_t[:], ow2.ap().rearrange("(kc p) c -> p kc c", p=128))
            ob2_t = wpool.tile([1, 1], F32)
            nc.sync.dma_start(ob2_t[:], ob2.ap())
            ones32_t = wpool.tile([1, 32], BF16)
            nc.vector.memset(ones32_t[:], 1.0)

            # ---- pools ----
            pgate = ctx.enter_context(tc.tile_pool(name="pgate", bufs=2, space="PSUM"))
            parena = ctx.enter_context(tc.tile_pool(name="parena", bufs=1, space="PSUM"))
            ppq = ctx.enter_context(tc.tile_pool(name="ppq", bufs=1, space="PSUM"))

            px = ctx.enter_context(tc.tile_pool(name="px", bufs=4))
            pch = ctx.enter_context(tc.tile_pool(name="pch", bufs=3))
            pst = ctx.enter_context(tc.tile_pool(name="pst", bufs=2))
            p2s = ctx.enter_context(tc.tile_pool(name="p2s", bufs=2))
            p2u = ctx.enter_context(tc.tile_pool(name="p2u", bufs=1))

            # ---- state ----
            h_cur = pst.tile([128, 3, 32], BF16, tag="h")
            nc.vector.memset(h_cur[:], 0.0)
            mem_cur = pst.tile([128, 2, 32], BF16, tag="mem")
            nc.vector.memset(mem_cur[:], 0.0)

            ch_tiles = {}     # chunk -> c_hist tile [128,3,9,32] F32R (C=2c)
            gate_tiles = {}   # group -> psum tile [128,12,4,32]
            x_tiles = {}      # group -> [128,4,128] F32R
            ug_tiles = {}     # chunk -> [128,4,256] F32 (normalized g L1 att-part)
            v_tiles = {}      # chunk -> [128,2,256] F32 (normalized att2 raw)
            et_t = {}
            ut_t = {}
            ug_last = [None]
            v_last = [None]

            def dma_x(G):
                if G >= NGRP:
                    return
                xt = px.tile([128, 4, 128], F32, tag="xt")
                c0 = G * GRP * B
                for kc in range(4):
                    rows = 128 if kc < 3 else DAUG - 384
                    nc.sync.dma_start(
                        xt[0:rows, kc, :],
                        xT.ap()[kc * 128:kc * 128 + rows, c0:c0 + GRP * B])
                x_tiles[G] = xt

            def p0_group(G):
                if G >= NGRP:
                    return
                xtf = x_tiles.pop(G)
                xt = px.tile([128, 4, 128], BF16, tag="xtb", name="xtb")
                nc.vector.tensor_copy(xt[:, 0:3, :], xtf[:, 0:3, :])
                nc.vector.tensor_copy(xt[0:DAUG - 384, 3, :], xtf[0:DAUG - 384, 3, :])
                gt = pgate.tile([128, 12, GRP, 32], F32, tag="gates")
                for s in range(12):
                    kcs = _nonzero_kcs(s)
                    for i, kc in enumerate(kcs):
                        rows = 128 if kc < 3 else DAUG - 384
                        # start=True only once per PSUM bank (s=0,4,8): start
                        # marks the whole 2KB zero-region pending; later slots
                        # land fresh on still-pending bytes.
                        nc.tensor.matmul(
                            gt[:, s, :, :], wihT_t[0:rows, kc, s * 128:(s + 1) * 128],
                            xt[0:rows, kc, :],
                            start=(i == 0 and s % 4 == 0), stop=(i == len(kcs) - 1),
                            skip_group_check=True)
                gate_tiles[G] = gt

            def cprev_ap(k, jj):
                if jj == 0:
                    if k == 0:
                        return ch_tiles[0][:, :, 0, :]
                    return ch_tiles[k - 1][:, :, 8, :]
                return ch_tiles[k][:, :, jj, :]

            def p1_step_pe(k, jj):
                G = (k * CH + jj) // GRP
                j = jj % GRP
                gt = gate_tiles[G]
                for s in range(12):
                    nc.tensor.matmul(
                        gt[:, s, j, :],
                        whhT_t[:, s * 128:(s + 1) * 128],
                        h_cur[:, s % 3, :],
                        start=False, stop=(s == 11), skip_group_check=True)

            def p1_step_rest(k, jj, act2=None, dve2=None):
                """ACT/DVE part of LSTM step; act2/dve2 are callbacks to issue
                interleaved p2a work inside the latency gaps."""
                nonlocal h_cur
                G = (k * CH + jj) // GRP
                j = jj % GRP
                gt = gate_tiles[G]
                chh = ch_tiles[k]
                tg9 = pst.tile([128, 12, 32], F32, tag="tg9")
                nc.scalar.activation(tg9[:], gt[:, :, j, :], AF.Tanh, scale=0.5)
                u2 = pst.tile([128, 3, 32], F32, tag="u2")
                nc.vector.scalar_tensor_tensor(
                    u2[:], tg9[:, 0:3, :], 1.0, tg9[:, 9:12, :], ALU.add, ALU.mult)
                u1 = pst.tile([128, 3, 32], F32, tag="u1")
                nc.vector.scalar_tensor_tensor(
                    u1[:], tg9[:, 3:6, :], 1.0, cprev_ap(k, jj),
                    ALU.add, ALU.mult)
                nc.vector.scalar_tensor_tensor(
                    chh[:, :, jj + 1, :], u1[:], 0.5, u2[:],
                    ALU.mult, ALU.add)
                if dve2 is not None:
                    dve2()
                tc_t = pst.tile([128, 3, 32], F32, tag="tc")
                nc.scalar.activation(tc_t[:], chh[:, :, jj + 1, :],
                                     AF.Tanh, scale=0.5)
                if act2 is not None:
                    act2()
                h_new = pst.tile([128, 3, 32], BF16, tag="h")
                nc.vector.scalar_tensor_tensor(
                    h_new[:], tg9[:, 6:9, :], 1.0, tc_t[:], ALU.add, ALU.mult)
                h_cur = h_new

            # ---------------- p2a slices ----------------
            def rhs_k(kk, q):
                chh = ch_tiles[kk]
                if q < 3:
                    return chh[:, q, 0:8, :]
                return chh[:, q - 3, 1:9, :]

            def p2a_slice(kk, slot):
                if slot == 0:
                    y1p = parena.tile([128, 2, 256], F32, tag="arena", name="y1p")
                    for mc in range(2):
                        for q in range(6):
                            nc.tensor.matmul(
                                y1p[:, mc, :], a1w1_t[:, q, mc * 128:(mc + 1) * 128],
                                rhs_k(kk, q), start=(q == 0), stop=(q == 5))
                    y1 = p2s.tile([128, 2, 256], BF16, tag="y1")
                    for mc in range(2):
                        nc.scalar.activation(y1[:, mc, :], y1p[:, mc, :],
                                             AF.Relu, bias=a1b1_t[:, mc:mc + 1])
                    p2a_slice.y1 = y1
                elif slot in (1, 2, 3):
                    hi = slot - 1
                    y1 = p2a_slice.y1
                    if hi == 0:
                        et_t[kk] = p2s.tile([128, 6, 256], BF16, tag="et", name="et")
                        ut_t[kk] = p2s.tile([128, 6, 256], BF16, tag="ut", name="ut")
                    ep = parena.tile([128, 2, 256], F32, tag="arena", name=f"ep{hi}")
                    for m2 in range(2):
                        mc6 = hi * 2 + m2
                        for kc in range(2):
                            nc.tensor.matmul(
                                ep[:, m2, :], a1w2_t[:, kc, mc6 * 128:(mc6 + 1) * 128],
                                y1[:, kc, :], start=(kc == 0), stop=(kc == 1))
                        nc.scalar.activation(
                            et_t[kk][:, mc6, :], ep[:, m2, :], AF.Exp,
                            bias=a1b2_t[:, mc6:mc6 + 1])
                        nc.vector.tensor_mul(
                            ut_t[kk][:, mc6, :], et_t[kk][:, mc6, :],
                            rhs_k(kk, mc6).rearrange("p t b -> p (t b)"))
                elif slot == 4:
                    et = et_t.pop(kk)
                    sp = parena.tile([128, 2, 256], F32, tag="arena", name="sp")
                    for q in range(6):
                        nc.tensor.matmul(sp[0:1, 0, :], ones128_t[:], et[:, q, :],
                                         start=(q == 0), stop=(q == 5))
                    srow = p2s.tile([1, 256], BF16, tag="srow")
                    nc.vector.tensor_copy(srow[:], sp[0:1, 0, :])
                    p2a_slice.srow = srow
                elif slot == 5:
                    sb = parena.tile([128, 2, 256], F32, tag="arena", name="sb")
                    nc.tensor.matmul(sb[:, 0, :], ones1x128_t[:], p2a_slice.srow[:],
                                     start=True, stop=True)
                    sbs = p2s.tile([128, 256], F32, tag="sbs")
                    nc.vector.tensor_copy(sbs[:], sb[:, 0, :])
                    sinvb = p2s.tile([128, 256], F32, tag="sinvb")
                    nc.vector.reciprocal_approx_fast(sinvb[:], sbs[:])
                    p2a_slice.sinvb = sinvb
                elif slot == 6:
                    ut = ut_t[kk]
                    srow = p2a_slice.srow
                    zp = parena.tile([128, 2, 256], F32, tag="arena", name="zp")
                    for mc in range(2):
                        for q in range(6):
                            nc.tensor.matmul(
                                zp[:, mc, :], a2w1_t[:, q, mc * 128:(mc + 1) * 128],
                                ut[:, q, :], start=(q == 0), stop=False)
                        nc.tensor.matmul(zp[:, mc, :], a2b1r_t[:, mc * 128:(mc + 1) * 128],
                                         srow[:], start=False, stop=True)
                    z = p2s.tile([128, 2, 256], BF16, tag="z")
                    for mc in range(2):
                        nc.scalar.activation(z[:, mc, :], zp[:, mc, :], AF.Relu)
                    p2a_slice.z = z
                elif slot == 7:
                    ut = ut_t.pop(kk)
                    srow = p2a_slice.srow
                    sinvb = p2a_slice.sinvb
                    z = p2a_slice.z
                    # att2 L2 (raw) + b2*S fold, then normalize
                    ap2 = parena.tile([128, 2, 256], F32, tag="arena", name="ap2")
                    for mc in range(2):
                        for kc in range(2):
                            nc.tensor.matmul(
                                ap2[:, mc, :], a2w2_t[:, kc, mc * 128:(mc + 1) * 128],
                                z[:, kc, :], start=(kc == 0), stop=False)
                        nc.tensor.matmul(ap2[:, mc, :], a2b2r_t[:, mc * 128:(mc + 1) * 128],
                                         srow[:], start=False, stop=True)
                    vt = p2s.tile([128, 2, 256], F32, tag="vt")
                    nc.vector.tensor_mul(
                        vt[:], ap2[:], sinvb[:].unsqueeze(1).broadcast_to([128, 2, 256]))
                    chc = p2s.tile([128, 2, 256], F32, tag="chc")
                    nc.scalar.activation(chc[:], vt[:], AF.Tanh)
                    v_tiles[kk] = chc
                    ug = p2s.tile([128, 4, 256], F32, tag="ug")
                    for gi, (gw, gbr) in enumerate(((g1a_t, g1b1r_t), (g2a_t, g2b1r_t))):
                        gp2 = parena.tile([128, 2, 256], F32, tag="arena", name=f"gp{gi}")
                        for mc in range(2):
                            for q in range(6):
                                nc.tensor.matmul(
                                    gp2[:, mc, :], gw[:, q, mc * 128:(mc + 1) * 128],
                                    ut[:, q, :], start=(q == 0), stop=False)
                            nc.tensor.matmul(gp2[:, mc, :], gbr[:, mc * 128:(mc + 1) * 128],
                                             srow[:], start=False, stop=True)
                        nc.vector.tensor_mul(
                            ug[:, gi * 2:gi * 2 + 2, :], gp2[:],
                            sinvb[:].unsqueeze(1).broadcast_to([128, 2, 256]))
                    ug_tiles[kk] = ug

            # ---------------- p3 ----------------
            def p3_step_pe1(kk, jj):
                pq = ppq.tile([128, 8, 32], F32, tag="pq", name="pq")
                p3_step_pe1.pq = pq
                pg = pq[:, 0:4, :]
                for r in range(4):
                    gwt = g1b_t if r < 2 else g2b_t
                    for kc in range(2):
                        nc.tensor.matmul(
                            pg[:, r, :], gwt[:, kc, (r % 2) * 128:(r % 2 + 1) * 128],
                            mem_cur[:, kc, :], start=(kc == 0), stop=(kc == 1))

            def p3_step_mid(kk, jj):
                pg = p3_step_pe1.pq[:, 0:4, :]
                ug = ug_tiles[kk]
                h1pre = pst.tile([128, 4, 32], F32, tag="h1pre")
                nc.vector.tensor_add(h1pre[:], ug[:, :, jj * 32:(jj + 1) * 32], pg[:])
                h1 = pst.tile([128, 4, 32], BF16, tag="h1")
                nc.scalar.activation(h1[:], h1pre[:], AF.Relu)
                p3_step_mid.h1 = h1

            def p3_step_pe2(kk, jj):
                h1 = p3_step_mid.h1
                qg = p3_step_pe1.pq[:, 4:8, :]
                for r in range(4):
                    nc.tensor.matmul(
                        qg[:, r, :], gb2r_t[0:1, r * 128:(r + 1) * 128], ones32_t[:],
                        start=(r == 0), stop=False, skip_group_check=True)
                for r in range(4):
                    gwt = g1w2_t if r < 2 else g2w2_t
                    goff = 0 if r < 2 else 2
                    for kc in range(2):
                        nc.tensor.matmul(
                            qg[:, r, :], gwt[:, kc, (r % 2) * 128:(r % 2 + 1) * 128],
                            h1[:, goff + kc, :], start=False,
                            stop=(r == 3 and kc == 1), skip_group_check=True)
                p3_step_pe2.qg = qg

            def p3_step_rest(kk, jj):
                nonlocal mem_cur
                qg = p3_step_pe2.qg
                chc = v_tiles[kk]
                tq = pst.tile([128, 4, 32], F32, tag="tq")
                nc.scalar.activation(tq[:], qg[:], AF.Tanh, scale=0.5)
                ua = pst.tile([128, 2, 32], F32, tag="ua")
                nc.vector.scalar_tensor_tensor(
                    ua[:], tq[:, 0:2, :], 1.0, mem_cur[:], ALU.add, ALU.mult)
                ub = pst.tile([128, 2, 32], F32, tag="ub")
                nc.vector.scalar_tensor_tensor(
                    ub[:], tq[:, 2:4, :], 1.0, chc[:, :, jj * 32:(jj + 1) * 32],
                    ALU.add, ALU.mult)
                mem_new = pst.tile([128, 2, 32], BF16, tag="mem")
                nc.vector.scalar_tensor_tensor(
                    mem_new[:], ua[:], 0.5, ub[:], ALU.mult, ALU.add)
                mem_cur = mem_new
                if jj == 7:
                    ug_last[0] = ug_tiles[kk]
                    v_last[0] = v_tiles[kk]
                    del ug_tiles[kk]
                    del v_tiles[kk]

            # ---------------- main pipeline ----------------
            dma_x(0)
            dma_x(1)
            p0_group(0)

            for k in range(NCH + 2):
                if k < NCH:
                    chh = pch.tile([128, 3, 9, 32], BF16, tag="chist", name="chist")
                    ch_tiles[k] = chh
                    if k == 0:
                        nc.vector.memset(chh[:], 0.0)
                    else:
                        # boundary slot: C_{t0-1} for window contiguity
                        nc.vector.tensor_copy(
                            chh[:, :, 0, :], ch_tiles[k - 1][:, :, 8, :])
                for g in range(2):
                    G = 2 * k + g
                    if k < NCH:
                        dma_x(G + 2)
                        p0_group(G + 1)
                    for j in range(GRP):
                        jj = g * GRP + j
                        kk2 = k - 1   # p2a chunk
                        kk3 = k - 2   # p3 chunk
                        # --- PE: p1 whh first (chain-critical) ---
                        if k < NCH:
                            p1_step_pe(k, jj)
                        # --- PE: p3 mem-part (depends on prev-step MEM) ---
                        if 0 <= kk3 < NCH:
                            p3_step_pe1(kk3, jj)
                        # --- p2a slice (bulk PE + ACT + DVE) ---
                        if 0 <= kk2 < NCH:
                            p2a_slice(kk2, jj)
                        # --- p3 mid (DVE add + ACT relu) then PE qg ---
                        if 0 <= kk3 < NCH:
                            p3_step_mid(kk3, jj)
                            p3_step_pe2(kk3, jj)
                        # --- p1 ACT/DVE chain ---
                        if k < NCH:
                            p1_step_rest(k, jj)
                        # --- p3 tail (ACT tanh + DVE stt) ---
                        if 0 <= kk3 < NCH:
                            p3_step_rest(kk3, jj)
                if k - 1 in ch_tiles and k >= 1 and k - 1 != NCH - 1:
                    del ch_tiles[k - 1]

            # ---------------- debug dumps ----------------
            hf32 = pst.tile([128, 3, 32], F32, tag="hf32")
            nc.vector.tensor_copy(hf32[:], h_cur[:])
            nc.sync.dma_start(dbg_h.ap(), hf32[:])
            nc.sync.dma_start(dbg_c.ap(), ch_tiles[NCH - 1][:, :, 8, :])
            mf32 = pst.tile([128, 2, 32], F32, tag="mf32")
            nc.vector.tensor_copy(mf32[:], mem_cur[:])
            nc.sync.dma_start(dbg_m.ap(), mf32[:])
            nc.sync.dma_start(dbg_ug.ap(), ug_last[0][:])
            nc.sync.dma_start(dbg_vt.ap(), v_last[0][:])
            if NCH == 1:
                nc.sync.dma_start(dbg_c0.ap(), ch_tiles[0][:])
                for Gd, dbg in ((0, dbg_g0), (1, dbg_g1)):
                    gev = pst.tile([128, 12, 4, 32], F32, tag="gev", name="gev")
                    nc.vector.tensor_copy(gev[:], gate_tiles[Gd][:])
                    nc.sync.dma_start(dbg.ap(), gev[:])

            # ---------------- phase 4: output MLP ----------------
            h_fin = h_cur
            opq = ppq.tile([128, 8, 32], F32, tag="pq", name="opq")
            o1p = opq[:, 0:4, :]
            rhs5 = [h_fin[:, 0, :], h_fin[:, 1, :], h_fin[:, 2, :],
                    mem_cur[:, 0, :], mem_cur[:, 1, :]]
            for mc in range(2):
                for kc in range(5):
                    nc.tensor.matmul(
                        o1p[:, mc, :], ow1_t[:, kc, mc * 128:(mc + 1) * 128],
                        rhs5[kc], start=(kc == 0), stop=(kc == 4))
            o1s = pst.tile([128, 2, 32], BF16, tag="o1s")
            for mc in range(2):
                nc.scalar.activation(o1s[:, mc, :], o1p[:, mc, :], AF.Relu,
                                     bias=ob1_t[:, mc:mc + 1])
            o2p = opq[:, 4:8, :]
            for kc in range(2):
                nc.tensor.matmul(o2p[0:1, 0, :], ow2_t[:, kc, :], o1s[:, kc, :],
                                 start=(kc == 0), stop=(kc == 1))
            o2s = pst.tile([1, 32], F32, tag="o2s")
            nc.scalar.activation(o2s[:], o2p[0:1, 0, :], AF.Identity, bias=ob2_t[:])
            nc.sync.dma_start(out_d.ap().rearrange("b one -> (one) (b)"), o2s[:])

    nc.compile()
    return nc


# ---------------------------------------------------------------------------
# host-side packing
# ---------------------------------------------------------------------------

def pack_shared(inp):
    f = np.float32
    bf = ml_dtypes.bfloat16
    d = {}
    wih = {0: inp["Wih_l"], 1: inp["Wih_a"], 2: inp["Wih_v"]}
    whh = {0: inp["Whh_l"], 1: inp["Whh_a"], 2: inp["Whh_v"]}
    bb = {m: (np.asarray(inp[f"bih_{k}"], f) + np.asarray(inp[f"bhh_{k}"], f))
          for m, k in ((0, "l"), (1, "a"), (2, "v"))}
    foff = {0: 0, 1: D_L, 2: D_L + D_A}
    din = {0: D_L, 1: D_A, 2: D_V}

    waug = np.zeros((512, 1536), f)
    whhT = np.zeros((128, 1536), f)
    for gq in range(4):
        tg = TORCH_G[gq]
        for m in range(3):
            s = gq * 3 + m
            wblk = np.asarray(wih[m], f)[tg * 128:(tg + 1) * 128, :]
            waug[foff[m]:foff[m] + din[m], s * 128:(s + 1) * 128] = wblk.T
            waug[DIN, s * 128:(s + 1) * 128] = bb[m][tg * 128:(tg + 1) * 128]
            whhT[:, s * 128:(s + 1) * 128] = np.asarray(whh[m], f)[tg * 128:(tg + 1) * 128, :].T
    # g~ slots (s=9,10,11) pre-scaled x2 so one tanh(0.5x) covers all gates
    for s in (9, 10, 11):
        waug[:, s * 128:(s + 1) * 128] *= 2.0
        whhT[:, s * 128:(s + 1) * 128] *= 2.0
    d["waug"] = waug.astype(bf)
    # h is stored doubled -> fold 0.5 into Whh
    d["whhT"] = (0.5 * whhT).astype(bf)
    d["ones128"] = np.ones((128, 1), bf)

    # cStar is stored doubled (C=2c) -> fold 0.5 into att1 W1
    d["a1w1"] = (0.5 * np.asarray(inp["att1_W1"], f).T).astype(bf)
    d["a1b1"] = np.asarray(inp["att1_b1"], f).reshape(2, 128).T.copy()
    d["a1w2"] = np.asarray(inp["att1_W2"], f).T.astype(bf)
    d["a1b2"] = np.asarray(inp["att1_b2"], f).reshape(6, 128).T.copy()
    d["a2w1"] = (0.5 * np.asarray(inp["att2_W1"], f).T).astype(bf)
    d["a2b1r"] = np.asarray(inp["att2_b1"], f).reshape(1, 256).astype(bf)
    d["a2w2"] = np.asarray(inp["att2_W2"], f).T.astype(bf)
    d["a2b2r"] = np.asarray(inp["att2_b2"], f).reshape(1, 256).astype(bf)
    d["g1a"] = (0.5 * np.asarray(inp["g1_W1"], f)[:, :768].T).astype(bf)
    d["g2a"] = (0.5 * np.asarray(inp["g2_W1"], f)[:, :768].T).astype(bf)
    # mem stored doubled (M=2mem) -> fold 0.5 into mem-part weights
    d["g1b"] = (0.5 * np.asarray(inp["g1_W1"], f)[:, 768:].T).astype(bf)
    d["g2b"] = (0.5 * np.asarray(inp["g2_W1"], f)[:, 768:].T).astype(bf)
    d["g1b1r"] = np.asarray(inp["g1_b1"], f).reshape(1, 256).astype(bf)
    d["g2b1r"] = np.asarray(inp["g2_b1"], f).reshape(1, 256).astype(bf)
    d["g1w2"] = np.asarray(inp["g1_W2"], f).T.astype(bf)
    d["g2w2"] = np.asarray(inp["g2_W2"], f).T.astype(bf)
    d["gb2r"] = np.concatenate(
        [np.asarray(inp["g1_b2"], f), np.asarray(inp["g2_b2"], f)]).reshape(1, 512).astype(bf)
    # out MLP consumes doubled h and doubled mem -> fold 0.5 everywhere
    d["ow1"] = (0.5 * np.asarray(inp["out_W1"], f).T).astype(bf)
    d["ob1"] = np.asarray(inp["out_b1"], f).reshape(2, 128).T.copy()
    d["ow2"] = np.asarray(inp["out_W2"], f).T.astype(bf)
    d["ob2"] = np.asarray(inp["out_b2"], f).reshape(1, 1).copy()
    return d


def pack_x(x, core, Tp):
    xc = np.asarray(x[:, core * B:(core + 1) * B, :], np.float32)
    xt = xc.transpose(2, 0, 1).reshape(DIN, Tp * B)
    return np.concatenate([xt, np.ones((1, Tp * B), np.float32)], 0)


_CACHE = {}


def _get_program(Tp):
    if Tp not in _CACHE:
        _CACHE[Tp] = build_program(Tp)
    return _CACHE[Tp]


def kernel(**inputs):
    x = np.asarray(inputs["x"])
    Tp = x.shape[0]
    nc = _get_program(Tp)
    shared = pack_shared({k: np.asarray(v) for k, v in inputs.items()})
    in_maps = []
    for c in range(NCORES):
        m = dict(shared)
        m["xT"] = np.ascontiguousarray(pack_x(x, c, Tp))
        in_maps.append(m)
    res = run_bass_kernel_spmd(nc, in_maps, list(range(NCORES))).results
    out = np.concatenate([r["out"] for r in res], axis=0)
    return out.astype(np.float32)


if __name__ == "__main__":
    import time
    t0 = time.time()
    nc = build_program(32)
    print("built in", time.time() - t0, "s")


# revision 7
# speedup vs baseline: 1.0488x; 1.0488x over previous
"""Trainium2 Bass kernel for nn_Contextual_MFN (Memory Fusion Network), v2.

Fully fused chunk pipeline (8-step chunks), SBUF/PSUM resident, no DRAM
spills.  Per step-slot the issue order interleaves:
  p1 (LSTM, chunk k)  |  p2a (attention, chunk k-1)  |  p3 (memory, chunk k-2)
with phase-0 (Wih@x -> gate PSUM) running one 4-step group ahead.

Tricks:
  - gates accumulate in PSUM: p0 f32r matmuls write xWb, p1 bf16 Whh matmuls
    accumulate start=False, ACT reads PSUM directly (no copy, no inject).
  - no sigmoid anywhere: carry C=2c, M=2mem; sigmoid(x) = 0.5*tanh(x/2)+0.5
    folds into scalar_tensor_tensor ops + host-side 0.5 weight scalings.
    Whole kernel uses {tanh, relu, exp, identity} = one ACT table set.
  - bf16 for all small serial matmuls (no fp32 LOW/HIGH split, FWL weight
    loads); attention path f32r at N=256 (full rate).
"""
import numpy as np
import ml_dtypes

import concourse.bass as bass
import concourse.tile as tile
from concourse import bacc, mybir
from concourse.bass_utils import run_bass_kernel_spmd

F32 = mybir.dt.float32
F32R = mybir.dt.float32r
BF16 = mybir.dt.bfloat16
AF = mybir.ActivationFunctionType
ALU = mybir.AluOpType

T_FULL = 512
NBATCH = 256
NCORES = 8
B = NBATCH // NCORES          # 32 batch rows per core
D_L, D_A, D_V = 300, 74, 35
DIN = D_L + D_A + D_V         # 409
DAUG = DIN + 1                # 410 (ones row for bias)
DH = 128
MEM = 256
CH = 8                        # chunk (steps)
GRP = 4                       # phase-0 gate group (steps)

# gate slot order: s = g'*3 + m, g' in (i, f, o, g_tanh); torch rows (i,f,g,o)
TORCH_G = (0, 1, 3, 2)


def _nonzero_kcs(s):
    m = s % 3
    if m == 0:
        return [0, 1, 2, 3]
    return [2, 3]


def build_program(Tp=T_FULL):
    assert Tp % CH == 0
    NCH = Tp // CH
    NGRP = Tp // GRP
    nc = bacc.Bacc("TRN2", target_bir_lowering=False, debug=False)

    # ---------------- external inputs ----------------
    xT = nc.dram_tensor("xT", [DAUG, Tp * B], F32, kind="ExternalInput")
    waug = nc.dram_tensor("waug", [512, 1536], BF16, kind="ExternalInput")
    whhT = nc.dram_tensor("whhT", [128, 1536], BF16, kind="ExternalInput")
    ones128 = nc.dram_tensor("ones128", [128, 1], BF16, kind="ExternalInput")

    a1w1 = nc.dram_tensor("a1w1", [768, 256], BF16, kind="ExternalInput")
    a1b1 = nc.dram_tensor("a1b1", [128, 2], F32, kind="ExternalInput")
    a1w2 = nc.dram_tensor("a1w2", [256, 768], BF16, kind="ExternalInput")
    a1b2 = nc.dram_tensor("a1b2", [128, 6], F32, kind="ExternalInput")
    a2w1 = nc.dram_tensor("a2w1", [768, 256], BF16, kind="ExternalInput")
    a2b1r = nc.dram_tensor("a2b1r", [1, 256], BF16, kind="ExternalInput")
    a2w2 = nc.dram_tensor("a2w2", [256, 256], BF16, kind="ExternalInput")
    a2b2r = nc.dram_tensor("a2b2r", [1, 256], BF16, kind="ExternalInput")
    g1a = nc.dram_tensor("g1a", [768, 256], BF16, kind="ExternalInput")
    g2a = nc.dram_tensor("g2a", [768, 256], BF16, kind="ExternalInput")
    g1b1r = nc.dram_tensor("g1b1r", [1, 256], BF16, kind="ExternalInput")
    g2b1r = nc.dram_tensor("g2b1r", [1, 256], BF16, kind="ExternalInput")
    g1b = nc.dram_tensor("g1b", [256, 256], BF16, kind="ExternalInput")
    g2b = nc.dram_tensor("g2b", [256, 256], BF16, kind="ExternalInput")
    g1w2 = nc.dram_tensor("g1w2", [256, 256], BF16, kind="ExternalInput")
    g2w2 = nc.dram_tensor("g2w2", [256, 256], BF16, kind="ExternalInput")
    gb2c = nc.dram_tensor("gb2c", [128, 4], F32, kind="ExternalInput")
    ow1 = nc.dram_tensor("ow1", [640, 256], BF16, kind="ExternalInput")
    ob1 = nc.dram_tensor("ob1", [128, 2], F32, kind="ExternalInput")
    ow2 = nc.dram_tensor("ow2", [256, 1], BF16, kind="ExternalInput")
    ob2 = nc.dram_tensor("ob2", [1, 1], F32, kind="ExternalInput")

    out_d = nc.dram_tensor("out", [B, 1], F32, kind="ExternalOutput")
    dbg_h = nc.dram_tensor("dbg_h", [128, 3, 32], F32, kind="ExternalOutput")
    dbg_c = nc.dram_tensor("dbg_c", [128, 3, 32], BF16, kind="ExternalOutput")
    dbg_m = nc.dram_tensor("dbg_m", [128, 2, 32], F32, kind="ExternalOutput")
    dbg_ug = nc.dram_tensor("dbg_ug", [128, 4, 256], F32, kind="ExternalOutput")
    dbg_vt = nc.dram_tensor("dbg_vt", [128, 2, 256], F32, kind="ExternalOutput")
    dbg_c0 = nc.dram_tensor("dbg_c0", [128, 3, 9, 32], BF16, kind="ExternalOutput")
    dbg_g0 = nc.dram_tensor("dbg_g0", [128, 12, 4, 32], F32, kind="ExternalOutput")
    dbg_g1 = nc.dram_tensor("dbg_g1", [128, 12, 4, 32], F32, kind="ExternalOutput")

    import contextlib
    with tile.TileContext(nc) as tc:
        ctx = contextlib.ExitStack()
        with ctx:
            wpool = ctx.enter_context(tc.tile_pool(name="weights", bufs=1))

            # ---- resident weights / constants ----
            wihT_t = wpool.tile([128, 4, 1536], BF16)
            nc.sync.dma_start(
                wihT_t[:], waug.ap().rearrange("(kc p) c -> p kc c", p=128))
            whhT_t = wpool.tile([128, 1536], BF16)
            nc.sync.dma_start(whhT_t[:], whhT.ap())
            ones128_t = wpool.tile([128, 1], BF16)
            nc.sync.dma_start(ones128_t[:], ones128.ap())
            ones1x128_t = wpool.tile([1, 128], BF16)
            nc.sync.dma_start(ones1x128_t[:], ones128.ap().rearrange("p one -> (one) (p)"))

            a1w1_t = wpool.tile([128, 6, 256], BF16)
            nc.sync.dma_start(a1w1_t[:], a1w1.ap().rearrange("(kc p) c -> p kc c", p=128))
            a1b1_t = wpool.tile([128, 2], F32)
            nc.sync.dma_start(a1b1_t[:], a1b1.ap())
            a1w2_t = wpool.tile([128, 2, 768], BF16)
            nc.sync.dma_start(a1w2_t[:], a1w2.ap().rearrange("(kc p) c -> p kc c", p=128))
            a1b2_t = wpool.tile([128, 6], F32)
            nc.sync.dma_start(a1b2_t[:], a1b2.ap())
            a2w1_t = wpool.tile([128, 6, 256], BF16)
            nc.sync.dma_start(a2w1_t[:], a2w1.ap().rearrange("(kc p) c -> p kc c", p=128))
            a2b1r_t = wpool.tile([1, 256], BF16)
            nc.sync.dma_start(a2b1r_t[:], a2b1r.ap())
            a2w2_t = wpool.tile([128, 2, 256], BF16)
            nc.sync.dma_start(a2w2_t[:], a2w2.ap().rearrange("(kc p) c -> p kc c", p=128))
            a2b2r_t = wpool.tile([1, 256], BF16)
            nc.sync.dma_start(a2b2r_t[:], a2b2r.ap())
            g1a_t = wpool.tile([128, 6, 256], BF16)
            nc.sync.dma_start(g1a_t[:], g1a.ap().rearrange("(kc p) c -> p kc c", p=128))
            g2a_t = wpool.tile([128, 6, 256], BF16)
            nc.sync.dma_start(g2a_t[:], g2a.ap().rearrange("(kc p) c -> p kc c", p=128))
            g1b1r_t = wpool.tile([1, 256], BF16)
            nc.sync.dma_start(g1b1r_t[:], g1b1r.ap())
            g2b1r_t = wpool.tile([1, 256], BF16)
            nc.sync.dma_start(g2b1r_t[:], g2b1r.ap())
            g1b_t = wpool.tile([128, 2, 256], BF16)
            nc.sync.dma_start(g1b_t[:], g1b.ap().rearrange("(kc p) c -> p kc c", p=128))
            g2b_t = wpool.tile([128, 2, 256], BF16)
            nc.sync.dma_start(g2b_t[:], g2b.ap().rearrange("(kc p) c -> p kc c", p=128))
            g1w2_t = wpool.tile([128, 2, 256], BF16)
            nc.sync.dma_start(g1w2_t[:], g1w2.ap().rearrange("(kc p) c -> p kc c", p=128))
            g2w2_t = wpool.tile([128, 2, 256], BF16)
            nc.sync.dma_start(g2w2_t[:], g2w2.ap().rearrange("(kc p) c -> p kc c", p=128))
            gb2c_t = wpool.tile([128, 4], F32)
            nc.sync.dma_start(gb2c_t[:], gb2c.ap())
            ow1_t = wpool.tile([128, 5, 256], BF16)
            nc.sync.dma_start(ow1_t[:], ow1.ap().rearrange("(kc p) c -> p kc c", p=128))
            ob1_t = wpool.tile([128, 2], F32)
            nc.sync.dma_start(ob1_t[:], ob1.ap())
            ow2_t = wpool.tile([128, 2, 1], BF16)
            nc.sync.dma_start(ow2_t[:], ow2.ap().rearrange("(kc p) c -> p kc c", p=128))
            ob2_t = wpool.tile([1, 1], F32)
            nc.sync.dma_start(ob2_t[:], ob2.ap())
            ones32_t = wpool.tile([1, 32], BF16)
            nc.vector.memset(ones32_t[:], 1.0)

            # ---- pools ----
            pgate = ctx.enter_context(tc.tile_pool(name="pgate", bufs=2, space="PSUM"))
            parena = ctx.enter_context(tc.tile_pool(name="parena", bufs=1, space="PSUM"))
            ppq = ctx.enter_context(tc.tile_pool(name="ppq", bufs=1, space="PSUM"))

            px = ctx.enter_context(tc.tile_pool(name="px", bufs=4))
            pch = ctx.enter_context(tc.tile_pool(name="pch", bufs=3))
            pst = ctx.enter_context(tc.tile_pool(name="pst", bufs=2))
            p2s = ctx.enter_context(tc.tile_pool(name="p2s", bufs=2))
            p2u = ctx.enter_context(tc.tile_pool(name="p2u", bufs=1))

            # ---- state ----
            h_cur = pst.tile([128, 3, 32], BF16, tag="h")
            nc.vector.memset(h_cur[:], 0.0)
            mem_cur = pst.tile([128, 2, 32], BF16, tag="mem")
            nc.vector.memset(mem_cur[:], 0.0)

            ch_tiles = {}     # chunk -> c_hist tile [128,3,9,32] F32R (C=2c)
            gate_tiles = {}   # group -> psum tile [128,12,4,32]
            x_tiles = {}      # group -> [128,4,128] F32R
            ug_tiles = {}     # chunk -> [128,4,256] F32 (normalized g L1 att-part)
            v_tiles = {}      # chunk -> [128,2,256] F32 (normalized att2 raw)
            et_t = {}
            ut_t = {}
            ug_last = [None]
            v_last = [None]

            def dma_x(G):
                if G >= NGRP:
                    return
                xt = px.tile([128, 4, 128], F32, tag="xt")
                c0 = G * GRP * B
                for kc in range(4):
                    rows = 128 if kc < 3 else DAUG - 384
                    nc.sync.dma_start(
                        xt[0:rows, kc, :],
                        xT.ap()[kc * 128:kc * 128 + rows, c0:c0 + GRP * B])
                x_tiles[G] = xt

            def p0_group(G):
                if G >= NGRP:
                    return
                xtf = x_tiles.pop(G)
                xt = px.tile([128, 4, 128], BF16, tag="xtb", name="xtb")
                nc.vector.tensor_copy(xt[:, 0:3, :], xtf[:, 0:3, :])
                nc.vector.tensor_copy(xt[0:DAUG - 384, 3, :], xtf[0:DAUG - 384, 3, :])
                gt = pgate.tile([128, 12, GRP, 32], F32, tag="gates")
                for s in range(12):
                    kcs = _nonzero_kcs(s)
                    for i, kc in enumerate(kcs):
                        rows = 128 if kc < 3 else DAUG - 384
                        # start=True only once per PSUM bank (s=0,4,8): start
                        # marks the whole 2KB zero-region pending; later slots
                        # land fresh on still-pending bytes.
                        nc.tensor.matmul(
                            gt[:, s, :, :], wihT_t[0:rows, kc, s * 128:(s + 1) * 128],
                            xt[0:rows, kc, :],
                            start=(i == 0 and s % 4 == 0), stop=(i == len(kcs) - 1),
                            skip_group_check=True)
                gate_tiles[G] = gt

            def cprev_ap(k, jj):
                if jj == 0:
                    if k == 0:
                        return ch_tiles[0][:, :, 0, :]
                    return ch_tiles[k - 1][:, :, 8, :]
                return ch_tiles[k][:, :, jj, :]

            def p1_step_pe(k, jj):
                G = (k * CH + jj) // GRP
                j = jj % GRP
                gt = gate_tiles[G]
                for s in range(12):
                    nc.tensor.matmul(
                        gt[:, s, j, :],
                        whhT_t[:, s * 128:(s + 1) * 128],
                        h_cur[:, s % 3, :],
                        start=False, stop=(s == 11), skip_group_check=True)

            def p1_step_rest(k, jj, act2=None, dve2=None):
                """ACT/DVE part of LSTM step; act2/dve2 are callbacks to issue
                interleaved p2a work inside the latency gaps."""
                nonlocal h_cur
                G = (k * CH + jj) // GRP
                j = jj % GRP
                gt = gate_tiles[G]
                chh = ch_tiles[k]
                tg9 = pst.tile([128, 12, 32], F32, tag="tg9")
                nc.scalar.activation(tg9[:], gt[:, :, j, :], AF.Tanh, scale=0.5)
                u2 = pst.tile([128, 3, 32], F32, tag="u2")
                nc.vector.scalar_tensor_tensor(
                    u2[:], tg9[:, 0:3, :], 1.0, tg9[:, 9:12, :], ALU.add, ALU.mult)
                u1 = pst.tile([128, 3, 32], F32, tag="u1")
                nc.vector.scalar_tensor_tensor(
                    u1[:], tg9[:, 3:6, :], 1.0, cprev_ap(k, jj),
                    ALU.add, ALU.mult)
                nc.vector.scalar_tensor_tensor(
                    chh[:, :, jj + 1, :], u1[:], 0.5, u2[:],
                    ALU.mult, ALU.add)
                if dve2 is not None:
                    dve2()
                tc_t = pst.tile([128, 3, 32], F32, tag="tc")
                nc.scalar.activation(tc_t[:], chh[:, :, jj + 1, :],
                                     AF.Tanh, scale=0.5)
                if act2 is not None:
                    act2()
                h_new = pst.tile([128, 3, 32], BF16, tag="h")
                nc.vector.scalar_tensor_tensor(
                    h_new[:], tg9[:, 6:9, :], 1.0, tc_t[:], ALU.add, ALU.mult)
                h_cur = h_new

            # ---------------- p2a slices ----------------
            def rhs_k(kk, q):
                chh = ch_tiles[kk]
                if q < 3:
                    return chh[:, q, 0:8, :]
                return chh[:, q - 3, 1:9, :]

            def p2a_slice(kk, slot):
                if slot == 0:
                    y1p = parena.tile([128, 2, 256], F32, tag="arena", name="y1p")
                    for mc in range(2):
                        for q in range(6):
                            nc.tensor.matmul(
                                y1p[:, mc, :], a1w1_t[:, q, mc * 128:(mc + 1) * 128],
                                rhs_k(kk, q), start=(q == 0), stop=(q == 5))
                    y1 = p2s.tile([128, 2, 256], BF16, tag="y1")
                    for mc in range(2):
                        nc.scalar.activation(y1[:, mc, :], y1p[:, mc, :],
                                             AF.Relu, bias=a1b1_t[:, mc:mc + 1])
                    p2a_slice.y1 = y1
                elif slot in (1, 2, 3):
                    hi = slot - 1
                    y1 = p2a_slice.y1
                    if hi == 0:
                        et_t[kk] = p2s.tile([128, 6, 256], BF16, tag="et", name="et")
                        ut_t[kk] = p2s.tile([128, 6, 256], BF16, tag="ut", name="ut")
                    ep = parena.tile([128, 2, 256], F32, tag="arena", name=f"ep{hi}")
                    for m2 in range(2):
                        mc6 = hi * 2 + m2
                        for kc in range(2):
                            nc.tensor.matmul(
                                ep[:, m2, :], a1w2_t[:, kc, mc6 * 128:(mc6 + 1) * 128],
                                y1[:, kc, :], start=(kc == 0), stop=(kc == 1))
                        nc.scalar.activation(
                            et_t[kk][:, mc6, :], ep[:, m2, :], AF.Exp,
                            bias=a1b2_t[:, mc6:mc6 + 1])
                        nc.vector.tensor_mul(
                            ut_t[kk][:, mc6, :], et_t[kk][:, mc6, :],
                            rhs_k(kk, mc6).rearrange("p t b -> p (t b)"))
                elif slot == 4:
                    et = et_t.pop(kk)
                    sp = parena.tile([128, 2, 256], F32, tag="arena", name="sp")
                    for q in range(6):
                        nc.tensor.matmul(sp[0:1, 0, :], ones128_t[:], et[:, q, :],
                                         start=(q == 0), stop=(q == 5))
                    srow = p2s.tile([1, 256], BF16, tag="srow")
                    nc.vector.tensor_copy(srow[:], sp[0:1, 0, :])
                    p2a_slice.srow = srow
                elif slot == 5:
                    sb = parena.tile([128, 2, 256], F32, tag="arena", name="sb")
                    nc.tensor.matmul(sb[:, 0, :], ones1x128_t[:], p2a_slice.srow[:],
                                     start=True, stop=True)
                    sbs = p2s.tile([128, 256], F32, tag="sbs")
                    nc.vector.tensor_copy(sbs[:], sb[:, 0, :])
                    sinvb = p2s.tile([128, 256], F32, tag="sinvb")
                    nc.vector.reciprocal_approx_fast(sinvb[:], sbs[:])
                    p2a_slice.sinvb = sinvb
                elif slot == 6:
                    ut = ut_t[kk]
                    srow = p2a_slice.srow
                    zp = parena.tile([128, 2, 256], F32, tag="arena", name="zp")
                    for mc in range(2):
                        for q in range(6):
                            nc.tensor.matmul(
                                zp[:, mc, :], a2w1_t[:, q, mc * 128:(mc + 1) * 128],
                                ut[:, q, :], start=(q == 0), stop=False)
                        nc.tensor.matmul(zp[:, mc, :], a2b1r_t[:, mc * 128:(mc + 1) * 128],
                                         srow[:], start=False, stop=True)
                    z = p2s.tile([128, 2, 256], BF16, tag="z")
                    for mc in range(2):
                        nc.scalar.activation(z[:, mc, :], zp[:, mc, :], AF.Relu)
                    p2a_slice.z = z
                elif slot == 7:
                    ut = ut_t.pop(kk)
                    srow = p2a_slice.srow
                    sinvb = p2a_slice.sinvb
                    z = p2a_slice.z
                    # att2 L2 (raw) + b2*S fold, then normalize
                    ap2 = parena.tile([128, 2, 256], F32, tag="arena", name="ap2")
                    for mc in range(2):
                        for kc in range(2):
                            nc.tensor.matmul(
                                ap2[:, mc, :], a2w2_t[:, kc, mc * 128:(mc + 1) * 128],
                                z[:, kc, :], start=(kc == 0), stop=False)
                        nc.tensor.matmul(ap2[:, mc, :], a2b2r_t[:, mc * 128:(mc + 1) * 128],
                                         srow[:], start=False, stop=True)
                    vt = p2s.tile([128, 2, 256], F32, tag="vt")
                    nc.vector.tensor_mul(
                        vt[:], ap2[:], sinvb[:].unsqueeze(1).broadcast_to([128, 2, 256]))
                    chc = p2s.tile([128, 2, 256], F32, tag="chc")
                    nc.scalar.activation(chc[:], vt[:], AF.Tanh)
                    v_tiles[kk] = chc
                    ug = p2s.tile([128, 4, 256], F32, tag="ug")
                    for gi, (gw, gbr) in enumerate(((g1a_t, g1b1r_t), (g2a_t, g2b1r_t))):
                        gp2 = parena.tile([128, 2, 256], F32, tag="arena", name=f"gp{gi}")
                        for mc in range(2):
                            for q in range(6):
                                nc.tensor.matmul(
                                    gp2[:, mc, :], gw[:, q, mc * 128:(mc + 1) * 128],
                                    ut[:, q, :], start=(q == 0), stop=False)
                            nc.tensor.matmul(gp2[:, mc, :], gbr[:, mc * 128:(mc + 1) * 128],
                                             srow[:], start=False, stop=True)
                        nc.vector.tensor_mul(
                            ug[:, gi * 2:gi * 2 + 2, :], gp2[:],
                            sinvb[:].unsqueeze(1).broadcast_to([128, 2, 256]))
                    ug_tiles[kk] = ug

            # ---------------- p3 ----------------
            def p3_step_pe1(kk, jj):
                pq = ppq.tile([128, 8, 32], F32, tag="pq", name="pq")
                p3_step_pe1.pq = pq
                pg = pq[:, 0:4, :]
                for r in range(4):
                    gwt = g1b_t if r < 2 else g2b_t
                    for kc in range(2):
                        nc.tensor.matmul(
                            pg[:, r, :], gwt[:, kc, (r % 2) * 128:(r % 2 + 1) * 128],
                            mem_cur[:, kc, :], start=(kc == 0), stop=(kc == 1))

            def p3_step_mid(kk, jj):
                pg = p3_step_pe1.pq[:, 0:4, :]
                ug = ug_tiles[kk]
                h1pre = pst.tile([128, 4, 32], F32, tag="h1pre")
                nc.vector.tensor_add(h1pre[:], ug[:, :, jj * 32:(jj + 1) * 32], pg[:])
                h1 = pst.tile([128, 4, 32], BF16, tag="h1")
                nc.scalar.activation(h1[:], h1pre[:], AF.Relu)
                p3_step_mid.h1 = h1

            def p3_step_pe2(kk, jj):
                h1 = p3_step_mid.h1
                qg = p3_step_pe1.pq[:, 4:8, :]
                for r in range(4):
                    gwt = g1w2_t if r < 2 else g2w2_t
                    goff = 0 if r < 2 else 2
                    for kc in range(2):
                        nc.tensor.matmul(
                            qg[:, r, :], gwt[:, kc, (r % 2) * 128:(r % 2 + 1) * 128],
                            h1[:, goff + kc, :], start=(r == 0 and kc == 0),
                            stop=(r == 3 and kc == 1), skip_group_check=True)
                p3_step_pe2.qg = qg

            def p3_step_rest(kk, jj):
                nonlocal mem_cur
                qg = p3_step_pe2.qg
                vt = v_tiles[kk]  # chc: tanh already applied per chunk
                tq = pst.tile([128, 4, 32], F32, tag="tq")
                for r in range(4):
                    nc.scalar.activation(tq[:, r, :], qg[:, r, :], AF.Tanh,
                                         scale=0.5, bias=gb2c_t[:, r:r + 1])
                ua = pst.tile([128, 2, 32], F32, tag="ua")
                nc.vector.scalar_tensor_tensor(
                    ua[:], tq[:, 0:2, :], 1.0, mem_cur[:], ALU.add, ALU.mult)
                ub = pst.tile([128, 2, 32], F32, tag="ub")
                nc.vector.scalar_tensor_tensor(
                    ub[:], tq[:, 2:4, :], 1.0, vt[:, :, jj * 32:(jj + 1) * 32],
                    ALU.add, ALU.mult)
                mem_new = pst.tile([128, 2, 32], BF16, tag="mem")
                nc.vector.scalar_tensor_tensor(
                    mem_new[:], ua[:], 0.5, ub[:], ALU.mult, ALU.add)
                mem_cur = mem_new
                if jj == 7:
                    ug_last[0] = ug_tiles[kk]
                    v_last[0] = v_tiles[kk]
                    del ug_tiles[kk]
                    del v_tiles[kk]

            # ---------------- main pipeline ----------------
            dma_x(0)
            dma_x(1)
            p0_group(0)

            for k in range(NCH + 2):
                if k < NCH:
                    chh = pch.tile([128, 3, 9, 32], BF16, tag="chist", name="chist")
                    ch_tiles[k] = chh
                    if k == 0:
                        nc.vector.memset(chh[:], 0.0)
                    else:
                        # boundary slot: C_{t0-1} for window contiguity
                        nc.vector.tensor_copy(
                            chh[:, :, 0, :], ch_tiles[k - 1][:, :, 8, :])
                for g in range(2):
                    G = 2 * k + g
                    if k < NCH:
                        dma_x(G + 2)
                        p0_group(G + 1)
                    for j in range(GRP):
                        jj = g * GRP + j
                        kk2 = k - 1   # p2a chunk
                        kk3 = k - 2   # p3 chunk
                        # --- PE: p1 whh first (chain-critical) ---
                        if k < NCH:
                            p1_step_pe(k, jj)
                        # --- PE: p3 mem-part (depends on prev-step MEM) ---
                        if 0 <= kk3 < NCH:
                            p3_step_pe1(kk3, jj)
                        # --- p2a slice (bulk PE + ACT + DVE) ---
                        if 0 <= kk2 < NCH:
                            p2a_slice(kk2, jj)
                        # --- p3 mid (DVE add + ACT relu) then PE qg ---
                        if 0 <= kk3 < NCH:
                            p3_step_mid(kk3, jj)
                            p3_step_pe2(kk3, jj)
                        # --- p1 ACT/DVE chain ---
                        if k < NCH:
                            p1_step_rest(k, jj)
                        # --- p3 tail (ACT tanh + DVE stt) ---
                        if 0 <= kk3 < NCH:
                            p3_step_rest(kk3, jj)
                if k - 1 in ch_tiles and k >= 1 and k - 1 != NCH - 1:
                    del ch_tiles[k - 1]

            # ---------------- debug dumps ----------------
            hf32 = pst.tile([128, 3, 32], F32, tag="hf32")
            nc.vector.tensor_copy(hf32[:], h_cur[:])
            nc.sync.dma_start(dbg_h.ap(), hf32[:])
            nc.sync.dma_start(dbg_c.ap(), ch_tiles[NCH - 1][:, :, 8, :])
            mf32 = pst.tile([128, 2, 32], F32, tag="mf32")
            nc.vector.tensor_copy(mf32[:], mem_cur[:])
            nc.sync.dma_start(dbg_m.ap(), mf32[:])
            nc.sync.dma_start(dbg_ug.ap(), ug_last[0][:])
            nc.sync.dma_start(dbg_vt.ap(), v_last[0][:])
            if NCH == 1:
                nc.sync.dma_start(dbg_c0.ap(), ch_tiles[0][:])
                for Gd, dbg in ((0, dbg_g0), (1, dbg_g1)):
                    gev = pst.tile([128, 12, 4, 32], F32, tag="gev", name="gev")
                    nc.vector.tensor_copy(gev[:], gate_tiles[Gd][:])
                    nc.sync.dma_start(dbg.ap(), gev[:])

            # ---------------- phase 4: output MLP ----------------
            h_fin = h_cur
            opq = ppq.tile([128, 8, 32], F32, tag="pq", name="opq")
            o1p = opq[:, 0:4, :]
            rhs5 = [h_fin[:, 0, :], h_fin[:, 1, :], h_fin[:, 2, :],
                    mem_cur[:, 0, :], mem_cur[:, 1, :]]
            for mc in range(2):
                for kc in range(5):
                    nc.tensor.matmul(
                        o1p[:, mc, :], ow1_t[:, kc, mc * 128:(mc + 1) * 128],
                        rhs5[kc], start=(kc == 0), stop=(kc == 4))
            o1s = pst.tile([128, 2, 32], BF16, tag="o1s")
            for mc in range(2):
                nc.scalar.activation(o1s[:, mc, :], o1p[:, mc, :], AF.Relu,
                                     bias=ob1_t[:, mc:mc + 1])
            o2p = opq[:, 4:8, :]
            for kc in range(2):
                nc.tensor.matmul(o2p[0:1, 0, :], ow2_t[:, kc, :], o1s[:, kc, :],
                                 start=(kc == 0), stop=(kc == 1))
            o2s = pst.tile([1, 32], F32, tag="o2s")
            nc.scalar.activation(o2s[:], o2p[0:1, 0, :], AF.Identity, bias=ob2_t[:])
            nc.sync.dma_start(out_d.ap().rearrange("b one -> (one) (b)"), o2s[:])

    nc.compile()
    return nc


# ---------------------------------------------------------------------------
# host-side packing
# ---------------------------------------------------------------------------

def pack_shared(inp):
    f = np.float32
    bf = ml_dtypes.bfloat16
    d = {}
    wih = {0: inp["Wih_l"], 1: inp["Wih_a"], 2: inp["Wih_v"]}
    whh = {0: inp["Whh_l"], 1: inp["Whh_a"], 2: inp["Whh_v"]}
    bb = {m: (np.asarray(inp[f"bih_{k}"], f) + np.asarray(inp[f"bhh_{k}"], f))
          for m, k in ((0, "l"), (1, "a"), (2, "v"))}
    foff = {0: 0, 1: D_L, 2: D_L + D_A}
    din = {0: D_L, 1: D_A, 2: D_V}

    waug = np.zeros((512, 1536), f)
    whhT = np.zeros((128, 1536), f)
    for gq in range(4):
        tg = TORCH_G[gq]
        for m in range(3):
            s = gq * 3 + m
            wblk = np.asarray(wih[m], f)[tg * 128:(tg + 1) * 128, :]
            waug[foff[m]:foff[m] + din[m], s * 128:(s + 1) * 128] = wblk.T
            waug[DIN, s * 128:(s + 1) * 128] = bb[m][tg * 128:(tg + 1) * 128]
            whhT[:, s * 128:(s + 1) * 128] = np.asarray(whh[m], f)[tg * 128:(tg + 1) * 128, :].T
    # g~ slots (s=9,10,11) pre-scaled x2 so one tanh(0.5x) covers all gates
    for s in (9, 10, 11):
        waug[:, s * 128:(s + 1) * 128] *= 2.0
        whhT[:, s * 128:(s + 1) * 128] *= 2.0
    d["waug"] = waug.astype(bf)
    # h is stored doubled -> fold 0.5 into Whh
    d["whhT"] = (0.5 * whhT).astype(bf)
    d["ones128"] = np.ones((128, 1), bf)

    # cStar is stored doubled (C=2c) -> fold 0.5 into att1 W1
    d["a1w1"] = (0.5 * np.asarray(inp["att1_W1"], f).T).astype(bf)
    d["a1b1"] = np.asarray(inp["att1_b1"], f).reshape(2, 128).T.copy()
    d["a1w2"] = np.asarray(inp["att1_W2"], f).T.astype(bf)
    d["a1b2"] = np.asarray(inp["att1_b2"], f).reshape(6, 128).T.copy()
    d["a2w1"] = (0.5 * np.asarray(inp["att2_W1"], f).T).astype(bf)
    d["a2b1r"] = np.asarray(inp["att2_b1"], f).reshape(1, 256).astype(bf)
    d["a2w2"] = np.asarray(inp["att2_W2"], f).T.astype(bf)
    d["a2b2r"] = np.asarray(inp["att2_b2"], f).reshape(1, 256).astype(bf)
    d["g1a"] = (0.5 * np.asarray(inp["g1_W1"], f)[:, :768].T).astype(bf)
    d["g2a"] = (0.5 * np.asarray(inp["g2_W1"], f)[:, :768].T).astype(bf)
    # mem stored doubled (M=2mem) -> fold 0.5 into mem-part weights
    d["g1b"] = (0.5 * np.asarray(inp["g1_W1"], f)[:, 768:].T).astype(bf)
    d["g2b"] = (0.5 * np.asarray(inp["g2_W1"], f)[:, 768:].T).astype(bf)
    d["g1b1r"] = np.asarray(inp["g1_b1"], f).reshape(1, 256).astype(bf)
    d["g2b1r"] = np.asarray(inp["g2_b1"], f).reshape(1, 256).astype(bf)
    d["g1w2"] = np.asarray(inp["g1_W2"], f).T.astype(bf)
    d["g2w2"] = np.asarray(inp["g2_W2"], f).T.astype(bf)
    d["gb2c"] = (0.5 * np.concatenate(
        [np.asarray(inp["g1_b2"], f), np.asarray(inp["g2_b2"], f)]).reshape(4, 128).T).copy()
    # out MLP consumes doubled h and doubled mem -> fold 0.5 everywhere
    d["ow1"] = (0.5 * np.asarray(inp["out_W1"], f).T).astype(bf)
    d["ob1"] = np.asarray(inp["out_b1"], f).reshape(2, 128).T.copy()
    d["ow2"] = np.asarray(inp["out_W2"], f).T.astype(bf)
    d["ob2"] = np.asarray(inp["out_b2"], f).reshape(1, 1).copy()
    return d


def pack_x(x, core, Tp):
    xc = np.asarray(x[:, core * B:(core + 1) * B, :], np.float32)
    xt = xc.transpose(2, 0, 1).reshape(DIN, Tp * B)
    return np.concatenate([xt, np.ones((1, Tp * B), np.float32)], 0)


_CACHE = {}


def _get_program(Tp):
    if Tp not in _CACHE:
        _CACHE[Tp] = build_program(Tp)
    return _CACHE[Tp]


def kernel(**inputs):
    x = np.asarray(inputs["x"])
    Tp = x.shape[0]
    nc = _get_program(Tp)
    shared = pack_shared({k: np.asarray(v) for k, v in inputs.items()})
    in_maps = []
    for c in range(NCORES):
        m = dict(shared)
        m["xT"] = np.ascontiguousarray(pack_x(x, c, Tp))
        in_maps.append(m)
    res = run_bass_kernel_spmd(nc, in_maps, list(range(NCORES))).results
    out = np.concatenate([r["out"] for r in res], axis=0)
    return out.astype(np.float32)


if __name__ == "__main__":
    import time
    t0 = time.time()
    nc = build_program(32)
    print("built in", time.time() - t0, "s")


# revision 9
# speedup vs baseline: 1.1072x; 1.0557x over previous
"""Trainium2 Bass kernel for nn_Contextual_MFN (Memory Fusion Network), v2.

Fully fused chunk pipeline (8-step chunks), SBUF/PSUM resident, no DRAM
spills.  Per step-slot the issue order interleaves:
  p1 (LSTM, chunk k)  |  p2a (attention, chunk k-1)  |  p3 (memory, chunk k-2)
with phase-0 (Wih@x -> gate PSUM) running one 4-step group ahead.

Tricks:
  - gates accumulate in PSUM: p0 f32r matmuls write xWb, p1 bf16 Whh matmuls
    accumulate start=False, ACT reads PSUM directly (no copy, no inject).
  - no sigmoid anywhere: carry C=2c, M=2mem; sigmoid(x) = 0.5*tanh(x/2)+0.5
    folds into scalar_tensor_tensor ops + host-side 0.5 weight scalings.
    Whole kernel uses {tanh, relu, exp, identity} = one ACT table set.
  - bf16 for all small serial matmuls (no fp32 LOW/HIGH split, FWL weight
    loads); attention path f32r at N=256 (full rate).
"""
import numpy as np
import ml_dtypes

import concourse.bass as bass
import concourse.tile as tile
from concourse import bacc, mybir
from concourse.bass_utils import run_bass_kernel_spmd

F32 = mybir.dt.float32
F32R = mybir.dt.float32r
BF16 = mybir.dt.bfloat16
AF = mybir.ActivationFunctionType
ALU = mybir.AluOpType

T_FULL = 512
NBATCH = 256
NCORES = 8
B = NBATCH // NCORES          # 32 batch rows per core
D_L, D_A, D_V = 300, 74, 35
DIN = D_L + D_A + D_V         # 409
DAUG = DIN + 1                # 410 (ones row for bias)
DH = 128
MEM = 256
CH = 8                        # chunk (steps)
GRP = 4                       # phase-0 gate group (steps)

# gate slot order: s = g'*3 + m, g' in (i, f, o, g_tanh); torch rows (i,f,g,o)
TORCH_G = (0, 1, 3, 2)


def _nonzero_kcs(s):
    m = s % 3
    if m == 0:
        return [0, 1, 2, 3]
    return [2, 3]


def build_program(Tp=T_FULL):
    assert Tp % CH == 0
    NCH = Tp // CH
    NGRP = Tp // GRP
    nc = bacc.Bacc("TRN2", target_bir_lowering=False, debug=False)

    # ---------------- external inputs ----------------
    xT = nc.dram_tensor("xT", [DAUG, Tp * B], F32, kind="ExternalInput")
    waug = nc.dram_tensor("waug", [512, 1536], BF16, kind="ExternalInput")
    whhT = nc.dram_tensor("whhT", [128, 1536], BF16, kind="ExternalInput")
    ones128 = nc.dram_tensor("ones128", [128, 1], BF16, kind="ExternalInput")

    a1w1 = nc.dram_tensor("a1w1", [768, 256], BF16, kind="ExternalInput")
    a1b1 = nc.dram_tensor("a1b1", [128, 2], F32, kind="ExternalInput")
    a1w2 = nc.dram_tensor("a1w2", [256, 768], BF16, kind="ExternalInput")
    a1b2 = nc.dram_tensor("a1b2", [128, 6], F32, kind="ExternalInput")
    a2w1 = nc.dram_tensor("a2w1", [768, 256], BF16, kind="ExternalInput")
    a2b1r = nc.dram_tensor("a2b1r", [1, 256], BF16, kind="ExternalInput")
    a2w2 = nc.dram_tensor("a2w2", [256, 256], BF16, kind="ExternalInput")
    a2b2r = nc.dram_tensor("a2b2r", [1, 256], BF16, kind="ExternalInput")
    g1a = nc.dram_tensor("g1a", [768, 256], BF16, kind="ExternalInput")
    g2a = nc.dram_tensor("g2a", [768, 256], BF16, kind="ExternalInput")
    g1b1r = nc.dram_tensor("g1b1r", [1, 256], BF16, kind="ExternalInput")
    g2b1r = nc.dram_tensor("g2b1r", [1, 256], BF16, kind="ExternalInput")
    g1b = nc.dram_tensor("g1b", [256, 256], BF16, kind="ExternalInput")
    g2b = nc.dram_tensor("g2b", [256, 256], BF16, kind="ExternalInput")
    g1w2 = nc.dram_tensor("g1w2", [256, 256], BF16, kind="ExternalInput")
    g2w2 = nc.dram_tensor("g2w2", [256, 256], BF16, kind="ExternalInput")
    gb2c = nc.dram_tensor("gb2c", [128, 4], F32, kind="ExternalInput")
    ow1 = nc.dram_tensor("ow1", [640, 256], BF16, kind="ExternalInput")
    ob1 = nc.dram_tensor("ob1", [128, 2], F32, kind="ExternalInput")
    ow2 = nc.dram_tensor("ow2", [256, 1], BF16, kind="ExternalInput")
    ob2 = nc.dram_tensor("ob2", [1, 1], F32, kind="ExternalInput")

    out_d = nc.dram_tensor("out", [B, 1], F32, kind="ExternalOutput")
    dbg_h = nc.dram_tensor("dbg_h", [128, 3, 32], F32, kind="ExternalOutput")
    dbg_c = nc.dram_tensor("dbg_c", [128, 3, 32], BF16, kind="ExternalOutput")
    dbg_m = nc.dram_tensor("dbg_m", [128, 2, 32], F32, kind="ExternalOutput")
    dbg_ug = nc.dram_tensor("dbg_ug", [128, 4, 256], F32, kind="ExternalOutput")
    dbg_vt = nc.dram_tensor("dbg_vt", [128, 2, 256], F32, kind="ExternalOutput")
    dbg_c0 = nc.dram_tensor("dbg_c0", [128, 3, 9, 32], BF16, kind="ExternalOutput")
    dbg_g0 = nc.dram_tensor("dbg_g0", [128, 12, 4, 32], F32, kind="ExternalOutput")
    dbg_g1 = nc.dram_tensor("dbg_g1", [128, 12, 4, 32], F32, kind="ExternalOutput")

    import contextlib
    with tile.TileContext(nc) as tc:
        ctx = contextlib.ExitStack()
        with ctx:
            wpool = ctx.enter_context(tc.tile_pool(name="weights", bufs=1))

            # ---- resident weights / constants ----
            wihT_t = wpool.tile([128, 4, 1536], BF16)
            nc.sync.dma_start(
                wihT_t[:], waug.ap().rearrange("(kc p) c -> p kc c", p=128))
            whhT_t = wpool.tile([128, 1536], BF16)
            nc.sync.dma_start(whhT_t[:], whhT.ap())
            ones128_t = wpool.tile([128, 1], BF16)
            nc.sync.dma_start(ones128_t[:], ones128.ap())
            ones1x128_t = wpool.tile([1, 128], BF16)
            nc.sync.dma_start(ones1x128_t[:], ones128.ap().rearrange("p one -> (one) (p)"))

            a1w1_t = wpool.tile([128, 6, 256], BF16)
            nc.sync.dma_start(a1w1_t[:], a1w1.ap().rearrange("(kc p) c -> p kc c", p=128))
            a1b1_t = wpool.tile([128, 2], F32)
            nc.sync.dma_start(a1b1_t[:], a1b1.ap())
            a1w2_t = wpool.tile([128, 2, 768], BF16)
            nc.sync.dma_start(a1w2_t[:], a1w2.ap().rearrange("(kc p) c -> p kc c", p=128))
            a1b2_t = wpool.tile([128, 6], F32)
            nc.sync.dma_start(a1b2_t[:], a1b2.ap())
            a2w1_t = wpool.tile([128, 6, 256], BF16)
            nc.sync.dma_start(a2w1_t[:], a2w1.ap().rearrange("(kc p) c -> p kc c", p=128))
            a2b1r_t = wpool.tile([1, 256], BF16)
            nc.sync.dma_start(a2b1r_t[:], a2b1r.ap())
            a2w2_t = wpool.tile([128, 2, 256], BF16)
            nc.sync.dma_start(a2w2_t[:], a2w2.ap().rearrange("(kc p) c -> p kc c", p=128))
            a2b2r_t = wpool.tile([1, 256], BF16)
            nc.sync.dma_start(a2b2r_t[:], a2b2r.ap())
            g1a_t = wpool.tile([128, 6, 256], BF16)
            nc.sync.dma_start(g1a_t[:], g1a.ap().rearrange("(kc p) c -> p kc c", p=128))
            g2a_t = wpool.tile([128, 6, 256], BF16)
            nc.sync.dma_start(g2a_t[:], g2a.ap().rearrange("(kc p) c -> p kc c", p=128))
            g1b1r_t = wpool.tile([1, 256], BF16)
            nc.sync.dma_start(g1b1r_t[:], g1b1r.ap())
            g2b1r_t = wpool.tile([1, 256], BF16)
            nc.sync.dma_start(g2b1r_t[:], g2b1r.ap())
            g1b_t = wpool.tile([128, 2, 256], BF16)
            nc.sync.dma_start(g1b_t[:], g1b.ap().rearrange("(kc p) c -> p kc c", p=128))
            g2b_t = wpool.tile([128, 2, 256], BF16)
            nc.sync.dma_start(g2b_t[:], g2b.ap().rearrange("(kc p) c -> p kc c", p=128))
            g1w2_t = wpool.tile([128, 2, 256], BF16)
            nc.sync.dma_start(g1w2_t[:], g1w2.ap().rearrange("(kc p) c -> p kc c", p=128))
            g2w2_t = wpool.tile([128, 2, 256], BF16)
            nc.sync.dma_start(g2w2_t[:], g2w2.ap().rearrange("(kc p) c -> p kc c", p=128))
            gb2c_t = wpool.tile([128, 4], F32)
            nc.sync.dma_start(gb2c_t[:], gb2c.ap())
            ow1_t = wpool.tile([128, 5, 256], BF16)
            nc.sync.dma_start(ow1_t[:], ow1.ap().rearrange("(kc p) c -> p kc c", p=128))
            ob1_t = wpool.tile([128, 2], F32)
            nc.sync.dma_start(ob1_t[:], ob1.ap())
            ow2_t = wpool.tile([128, 2, 1], BF16)
            nc.sync.dma_start(ow2_t[:], ow2.ap().rearrange("(kc p) c -> p kc c", p=128))
            ob2_t = wpool.tile([1, 1], F32)
            nc.sync.dma_start(ob2_t[:], ob2.ap())
            ones32_t = wpool.tile([1, 32], BF16)
            nc.vector.memset(ones32_t[:], 1.0)

            # ---- pools ----
            pgate = ctx.enter_context(tc.tile_pool(name="pgate", bufs=2, space="PSUM"))
            parena = ctx.enter_context(tc.tile_pool(name="parena", bufs=1, space="PSUM"))
            ppq = ctx.enter_context(tc.tile_pool(name="ppq", bufs=1, space="PSUM"))

            px = ctx.enter_context(tc.tile_pool(name="px", bufs=6))
            pch = ctx.enter_context(tc.tile_pool(name="pch", bufs=4))
            pst = ctx.enter_context(tc.tile_pool(name="pst", bufs=4))
            p2s = ctx.enter_context(tc.tile_pool(name="p2s", bufs=3))
            p2u = ctx.enter_context(tc.tile_pool(name="p2u", bufs=1))

            # ---- state ----
            h_cur = pst.tile([128, 3, 32], BF16, tag="h")
            nc.vector.memset(h_cur[:], 0.0)
            mem_cur = pst.tile([128, 2, 32], BF16, tag="mem")
            nc.vector.memset(mem_cur[:], 0.0)

            ch_tiles = {}     # chunk -> c_hist tile [128,3,9,32] F32R (C=2c)
            gate_tiles = {}   # group -> psum tile [128,12,4,32]
            x_tiles = {}      # group -> [128,4,128] F32R
            ug_tiles = {}     # chunk -> [128,4,256] F32 (normalized g L1 att-part)
            v_tiles = {}      # chunk -> [128,2,256] F32 (normalized att2 raw)
            et_t = {}
            ut_t = {}
            ug_last = [None]
            v_last = [None]

            def dma_x(G):
                if G >= NGRP:
                    return
                xt = px.tile([128, 4, 128], F32, tag="xt")
                c0 = G * GRP * B
                for kc in range(4):
                    rows = 128 if kc < 3 else DAUG - 384
                    nc.sync.dma_start(
                        xt[0:rows, kc, :],
                        xT.ap()[kc * 128:kc * 128 + rows, c0:c0 + GRP * B])
                x_tiles[G] = xt

            def p0_group(G):
                if G >= NGRP:
                    return
                xtf = x_tiles.pop(G)
                xt = px.tile([128, 4, 128], BF16, tag="xtb", name="xtb")
                nc.vector.tensor_copy(xt[:, 0:3, :], xtf[:, 0:3, :])
                nc.vector.tensor_copy(xt[0:DAUG - 384, 3, :], xtf[0:DAUG - 384, 3, :])
                gt = pgate.tile([128, 12, GRP, 32], F32, tag="gates")
                for s in range(12):
                    kcs = _nonzero_kcs(s)
                    for i, kc in enumerate(kcs):
                        rows = 128 if kc < 3 else DAUG - 384
                        # start=True only once per PSUM bank (s=0,4,8): start
                        # marks the whole 2KB zero-region pending; later slots
                        # land fresh on still-pending bytes.
                        nc.tensor.matmul(
                            gt[:, s, :, :], wihT_t[0:rows, kc, s * 128:(s + 1) * 128],
                            xt[0:rows, kc, :],
                            start=(i == 0 and s % 4 == 0), stop=(i == len(kcs) - 1),
                            skip_group_check=True)
                gate_tiles[G] = gt

            def cprev_ap(k, jj):
                if jj == 0:
                    if k == 0:
                        return ch_tiles[0][:, :, 0, :]
                    return ch_tiles[k - 1][:, :, 8, :]
                return ch_tiles[k][:, :, jj, :]

            def p1_step_pe(k, jj):
                G = (k * CH + jj) // GRP
                j = jj % GRP
                gt = gate_tiles[G]
                for s in range(12):
                    nc.tensor.matmul(
                        gt[:, s, j, :],
                        whhT_t[:, s * 128:(s + 1) * 128],
                        h_cur[:, s % 3, :],
                        start=False, stop=(s == 11), skip_group_check=True)

            def p1_step_rest(k, jj, act2=None, dve2=None):
                """ACT/DVE part of LSTM step; act2/dve2 are callbacks to issue
                interleaved p2a work inside the latency gaps."""
                nonlocal h_cur
                G = (k * CH + jj) // GRP
                j = jj % GRP
                gt = gate_tiles[G]
                chh = ch_tiles[k]
                tg9 = pst.tile([128, 12, 32], F32, tag="tg9")
                nc.scalar.activation(tg9[:], gt[:, :, j, :], AF.Tanh, scale=0.5)
                u2 = pst.tile([128, 3, 32], F32, tag="u2")
                nc.vector.scalar_tensor_tensor(
                    u2[:], tg9[:, 0:3, :], 1.0, tg9[:, 9:12, :], ALU.add, ALU.mult)
                u1 = pst.tile([128, 3, 32], F32, tag="u1")
                nc.vector.scalar_tensor_tensor(
                    u1[:], tg9[:, 3:6, :], 1.0, cprev_ap(k, jj),
                    ALU.add, ALU.mult)
                nc.vector.scalar_tensor_tensor(
                    chh[:, :, jj + 1, :], u1[:], 0.5, u2[:],
                    ALU.mult, ALU.add)
                if dve2 is not None:
                    dve2()
                tc_t = pst.tile([128, 3, 32], F32, tag="tc")
                nc.scalar.activation(tc_t[:], chh[:, :, jj + 1, :],
                                     AF.Tanh, scale=0.5)
                if act2 is not None:
                    act2()
                h_new = pst.tile([128, 3, 32], BF16, tag="h")
                nc.vector.scalar_tensor_tensor(
                    h_new[:], tg9[:, 6:9, :], 1.0, tc_t[:], ALU.add, ALU.mult)
                h_cur = h_new

            # ---------------- p2a slices ----------------
            def rhs_k(kk, q):
                chh = ch_tiles[kk]
                if q < 3:
                    return chh[:, q, 0:8, :]
                return chh[:, q - 3, 1:9, :]

            def p2a_slice(kk, slot):
                if slot == 0:
                    y1p = parena.tile([128, 2, 256], F32, tag="arena", name="y1p")
                    for mc in range(2):
                        for q in range(6):
                            nc.tensor.matmul(
                                y1p[:, mc, :], a1w1_t[:, q, mc * 128:(mc + 1) * 128],
                                rhs_k(kk, q), start=(q == 0), stop=(q == 5))
                    y1 = p2s.tile([128, 2, 256], BF16, tag="y1")
                    for mc in range(2):
                        nc.scalar.activation(y1[:, mc, :], y1p[:, mc, :],
                                             AF.Relu, bias=a1b1_t[:, mc:mc + 1])
                    p2a_slice.y1 = y1
                elif slot in (1, 2, 3):
                    hi = slot - 1
                    y1 = p2a_slice.y1
                    if hi == 0:
                        et_t[kk] = p2s.tile([128, 6, 256], BF16, tag="et", name="et")
                        ut_t[kk] = p2s.tile([128, 6, 256], BF16, tag="ut", name="ut")
                    ep = parena.tile([128, 2, 256], F32, tag="arena", name=f"ep{hi}")
                    for m2 in range(2):
                        mc6 = hi * 2 + m2
                        for kc in range(2):
                            nc.tensor.matmul(
                                ep[:, m2, :], a1w2_t[:, kc, mc6 * 128:(mc6 + 1) * 128],
                                y1[:, kc, :], start=(kc == 0), stop=(kc == 1))
                        nc.scalar.activation(
                            et_t[kk][:, mc6, :], ep[:, m2, :], AF.Exp,
                            bias=a1b2_t[:, mc6:mc6 + 1])
                        nc.vector.tensor_mul(
                            ut_t[kk][:, mc6, :], et_t[kk][:, mc6, :],
                            rhs_k(kk, mc6).rearrange("p t b -> p (t b)"))
                elif slot == 4:
                    et = et_t.pop(kk)
                    sp = parena.tile([128, 2, 256], F32, tag="arena", name="sp")
                    for q in range(6):
                        nc.tensor.matmul(sp[0:1, 0, :], ones128_t[:], et[:, q, :],
                                         start=(q == 0), stop=(q == 5))
                    srow = p2s.tile([1, 256], BF16, tag="srow")
                    nc.vector.tensor_copy(srow[:], sp[0:1, 0, :])
                    p2a_slice.srow = srow
                elif slot == 5:
                    sb = parena.tile([128, 2, 256], F32, tag="arena", name="sb")
                    nc.tensor.matmul(sb[:, 0, :], ones1x128_t[:], p2a_slice.srow[:],
                                     start=True, stop=True)
                    sbs = p2s.tile([128, 256], F32, tag="sbs")
                    nc.vector.tensor_copy(sbs[:], sb[:, 0, :])
                    sinvb = p2s.tile([128, 256], F32, tag="sinvb")
                    nc.vector.reciprocal_approx_fast(sinvb[:], sbs[:])
                    p2a_slice.sinvb = sinvb
                elif slot == 6:
                    ut = ut_t[kk]
                    srow = p2a_slice.srow
                    zp = parena.tile([128, 2, 256], F32, tag="arena", name="zp")
                    for mc in range(2):
                        for q in range(6):
                            nc.tensor.matmul(
                                zp[:, mc, :], a2w1_t[:, q, mc * 128:(mc + 1) * 128],
                                ut[:, q, :], start=(q == 0), stop=False)
                        nc.tensor.matmul(zp[:, mc, :], a2b1r_t[:, mc * 128:(mc + 1) * 128],
                                         srow[:], start=False, stop=True)
                    z = p2s.tile([128, 2, 256], BF16, tag="z")
                    for mc in range(2):
                        nc.scalar.activation(z[:, mc, :], zp[:, mc, :], AF.Relu)
                    p2a_slice.z = z
                elif slot == 7:
                    ut = ut_t.pop(kk)
                    srow = p2a_slice.srow
                    sinvb = p2a_slice.sinvb
                    z = p2a_slice.z
                    # att2 L2 (raw) + b2*S fold, then normalize
                    ap2 = parena.tile([128, 2, 256], F32, tag="arena", name="ap2")
                    for mc in range(2):
                        for kc in range(2):
                            nc.tensor.matmul(
                                ap2[:, mc, :], a2w2_t[:, kc, mc * 128:(mc + 1) * 128],
                                z[:, kc, :], start=(kc == 0), stop=False)
                        nc.tensor.matmul(ap2[:, mc, :], a2b2r_t[:, mc * 128:(mc + 1) * 128],
                                         srow[:], start=False, stop=True)
                    vt = p2s.tile([128, 2, 256], F32, tag="vt")
                    nc.vector.tensor_mul(
                        vt[:], ap2[:], sinvb[:].unsqueeze(1).broadcast_to([128, 2, 256]))
                    v_tiles[kk] = vt
                    ug = p2s.tile([128, 4, 256], F32, tag="ug")
                    for gi, (gw, gbr) in enumerate(((g1a_t, g1b1r_t), (g2a_t, g2b1r_t))):
                        gp2 = parena.tile([128, 2, 256], F32, tag="arena", name=f"gp{gi}")
                        for mc in range(2):
                            for q in range(6):
                                nc.tensor.matmul(
                                    gp2[:, mc, :], gw[:, q, mc * 128:(mc + 1) * 128],
                                    ut[:, q, :], start=(q == 0), stop=False)
                            nc.tensor.matmul(gp2[:, mc, :], gbr[:, mc * 128:(mc + 1) * 128],
                                             srow[:], start=False, stop=True)
                        nc.vector.tensor_mul(
                            ug[:, gi * 2:gi * 2 + 2, :], gp2[:],
                            sinvb[:].unsqueeze(1).broadcast_to([128, 2, 256]))
                    ug_tiles[kk] = ug

            # ---------------- p3 ----------------
            def p3_step_pe1(kk, jj):
                pq = ppq.tile([128, 8, 32], F32, tag="pq", name="pq")
                p3_step_pe1.pq = pq
                pg = pq[:, 0:4, :]
                for r in range(4):
                    gwt = g1b_t if r < 2 else g2b_t
                    for kc in range(2):
                        nc.tensor.matmul(
                            pg[:, r, :], gwt[:, kc, (r % 2) * 128:(r % 2 + 1) * 128],
                            mem_cur[:, kc, :], start=(kc == 0), stop=(kc == 1))

            def p3_step_mid(kk, jj):
                pg = p3_step_pe1.pq[:, 0:4, :]
                ug = ug_tiles[kk]
                h1pre = pst.tile([128, 4, 32], F32, tag="h1pre")
                nc.vector.tensor_add(h1pre[:], ug[:, :, jj * 32:(jj + 1) * 32], pg[:])
                h1 = pst.tile([128, 4, 32], BF16, tag="h1")
                nc.scalar.activation(h1[:], h1pre[:], AF.Relu)
                p3_step_mid.h1 = h1

            def p3_step_pe2(kk, jj):
                h1 = p3_step_mid.h1
                qg = p3_step_pe1.pq[:, 4:8, :]
                for r in range(4):
                    gwt = g1w2_t if r < 2 else g2w2_t
                    goff = 0 if r < 2 else 2
                    for kc in range(2):
                        nc.tensor.matmul(
                            qg[:, r, :], gwt[:, kc, (r % 2) * 128:(r % 2 + 1) * 128],
                            h1[:, goff + kc, :], start=(r == 0 and kc == 0),
                            stop=(r == 3 and kc == 1), skip_group_check=True)
                p3_step_pe2.qg = qg

            def p3_step_rest(kk, jj):
                nonlocal mem_cur
                qg = p3_step_pe2.qg
                vt = v_tiles[kk]
                tq = pst.tile([128, 4, 32], F32, tag="tq")
                for r in range(4):
                    nc.scalar.activation(tq[:, r, :], qg[:, r, :], AF.Tanh,
                                         scale=0.5, bias=gb2c_t[:, r:r + 1])
                cht = pst.tile([128, 2, 32], F32, tag="cht")
                nc.scalar.activation(cht[:], vt[:, :, jj * 32:(jj + 1) * 32], AF.Tanh)
                ua = pst.tile([128, 2, 32], F32, tag="ua")
                nc.vector.scalar_tensor_tensor(
                    ua[:], tq[:, 0:2, :], 1.0, mem_cur[:], ALU.add, ALU.mult)
                ub = pst.tile([128, 2, 32], F32, tag="ub")
                nc.vector.scalar_tensor_tensor(
                    ub[:], tq[:, 2:4, :], 1.0, cht[:], ALU.add, ALU.mult)
                mem_new = pst.tile([128, 2, 32], BF16, tag="mem")
                nc.vector.scalar_tensor_tensor(
                    mem_new[:], ua[:], 0.5, ub[:], ALU.mult, ALU.add)
                mem_cur = mem_new
                if jj == 7:
                    ug_last[0] = ug_tiles[kk]
                    v_last[0] = v_tiles[kk]
                    del ug_tiles[kk]
                    del v_tiles[kk]

            # ---------------- main pipeline ----------------
            dma_x(0)
            dma_x(1)
            p0_group(0)

            for k in range(NCH + 2):
                if k < NCH:
                    chh = pch.tile([128, 3, 9, 32], BF16, tag="chist", name="chist")
                    ch_tiles[k] = chh
                    if k == 0:
                        nc.vector.memset(chh[:], 0.0)
                    else:
                        # boundary slot: C_{t0-1} for window contiguity
                        nc.vector.tensor_copy(
                            chh[:, :, 0, :], ch_tiles[k - 1][:, :, 8, :])
                for g in range(2):
                    G = 2 * k + g
                    if k < NCH:
                        dma_x(G + 2)
                        p0_group(G + 1)
                    for j in range(GRP):
                        jj = g * GRP + j
                        kk2 = k - 1   # p2a chunk
                        kk3 = k - 2   # p3 chunk
                        # --- PE: p1 whh first (chain-critical) ---
                        if k < NCH:
                            p1_step_pe(k, jj)
                        # --- PE: p3 mem-part (depends on prev-step MEM) ---
                        if 0 <= kk3 < NCH:
                            p3_step_pe1(kk3, jj)
                        # --- p2a slice (bulk PE + ACT + DVE) ---
                        if 0 <= kk2 < NCH:
                            p2a_slice(kk2, jj)
                        # --- p3 mid (DVE add + ACT relu) then PE qg ---
                        if 0 <= kk3 < NCH:
                            p3_step_mid(kk3, jj)
                            p3_step_pe2(kk3, jj)
                        # --- p1 ACT/DVE chain ---
                        if k < NCH:
                            p1_step_rest(k, jj)
                        # --- p3 tail (ACT tanh + DVE stt) ---
                        if 0 <= kk3 < NCH:
                            p3_step_rest(kk3, jj)
                if k - 1 in ch_tiles and k >= 1 and k - 1 != NCH - 1:
                    del ch_tiles[k - 1]

            # ---------------- debug dumps ----------------
            hf32 = pst.tile([128, 3, 32], F32, tag="hf32")
            nc.vector.tensor_copy(hf32[:], h_cur[:])
            nc.sync.dma_start(dbg_h.ap(), hf32[:])
            nc.sync.dma_start(dbg_c.ap(), ch_tiles[NCH - 1][:, :, 8, :])
            mf32 = pst.tile([128, 2, 32], F32, tag="mf32")
            nc.vector.tensor_copy(mf32[:], mem_cur[:])
            nc.sync.dma_start(dbg_m.ap(), mf32[:])
            nc.sync.dma_start(dbg_ug.ap(), ug_last[0][:])
            nc.sync.dma_start(dbg_vt.ap(), v_last[0][:])
            if NCH == 1:
                nc.sync.dma_start(dbg_c0.ap(), ch_tiles[0][:])
                for Gd, dbg in ((0, dbg_g0), (1, dbg_g1)):
                    gev = pst.tile([128, 12, 4, 32], F32, tag="gev", name="gev")
                    nc.vector.tensor_copy(gev[:], gate_tiles[Gd][:])
                    nc.sync.dma_start(dbg.ap(), gev[:])

            # ---------------- phase 4: output MLP ----------------
            h_fin = h_cur
            opq = ppq.tile([128, 8, 32], F32, tag="pq", name="opq")
            o1p = opq[:, 0:4, :]
            rhs5 = [h_fin[:, 0, :], h_fin[:, 1, :], h_fin[:, 2, :],
                    mem_cur[:, 0, :], mem_cur[:, 1, :]]
            for mc in range(2):
                for kc in range(5):
                    nc.tensor.matmul(
                        o1p[:, mc, :], ow1_t[:, kc, mc * 128:(mc + 1) * 128],
                        rhs5[kc], start=(kc == 0), stop=(kc == 4))
            o1s = pst.tile([128, 2, 32], BF16, tag="o1s")
            for mc in range(2):
                nc.scalar.activation(o1s[:, mc, :], o1p[:, mc, :], AF.Relu,
                                     bias=ob1_t[:, mc:mc + 1])
            o2p = opq[:, 4:8, :]
            for kc in range(2):
                nc.tensor.matmul(o2p[0:1, 0, :], ow2_t[:, kc, :], o1s[:, kc, :],
                                 start=(kc == 0), stop=(kc == 1))
            o2s = pst.tile([1, 32], F32, tag="o2s")
            nc.scalar.activation(o2s[:], o2p[0:1, 0, :], AF.Identity, bias=ob2_t[:])
            nc.sync.dma_start(out_d.ap().rearrange("b one -> (one) (b)"), o2s[:])

    nc.compile()
    return nc


# ---------------------------------------------------------------------------
# host-side packing
# ---------------------------------------------------------------------------

def pack_shared(inp):
    f = np.float32
    bf = ml_dtypes.bfloat16
    d = {}
    wih = {0: inp["Wih_l"], 1: inp["Wih_a"], 2: inp["Wih_v"]}
    whh = {0: inp["Whh_l"], 1: inp["Whh_a"], 2: inp["Whh_v"]}
    bb = {m: (np.asarray(inp[f"bih_{k}"], f) + np.asarray(inp[f"bhh_{k}"], f))
          for m, k in ((0, "l"), (1, "a"), (2, "v"))}
    foff = {0: 0, 1: D_L, 2: D_L + D_A}
    din = {0: D_L, 1: D_A, 2: D_V}

    waug = np.zeros((512, 1536), f)
    whhT = np.zeros((128, 1536), f)
    for gq in range(4):
        tg = TORCH_G[gq]
        for m in range(3):
            s = gq * 3 + m
            wblk = np.asarray(wih[m], f)[tg * 128:(tg + 1) * 128, :]
            waug[foff[m]:foff[m] + din[m], s * 128:(s + 1) * 128] = wblk.T
            waug[DIN, s * 128:(s + 1) * 128] = bb[m][tg * 128:(tg + 1) * 128]
            whhT[:, s * 128:(s + 1) * 128] = np.asarray(whh[m], f)[tg * 128:(tg + 1) * 128, :].T
    # g~ slots (s=9,10,11) pre-scaled x2 so one tanh(0.5x) covers all gates
    for s in (9, 10, 11):
        waug[:, s * 128:(s + 1) * 128] *= 2.0
        whhT[:, s * 128:(s + 1) * 128] *= 2.0
    d["waug"] = waug.astype(bf)
    # h is stored doubled -> fold 0.5 into Whh
    d["whhT"] = (0.5 * whhT).astype(bf)
    d["ones128"] = np.ones((128, 1), bf)

    # cStar is stored doubled (C=2c) -> fold 0.5 into att1 W1
    d["a1w1"] = (0.5 * np.asarray(inp["att1_W1"], f).T).astype(bf)
    d["a1b1"] = np.asarray(inp["att1_b1"], f).reshape(2, 128).T.copy()
    d["a1w2"] = np.asarray(inp["att1_W2"], f).T.astype(bf)
    d["a1b2"] = np.asarray(inp["att1_b2"], f).reshape(6, 128).T.copy()
    d["a2w1"] = (0.5 * np.asarray(inp["att2_W1"], f).T).astype(bf)
    d["a2b1r"] = np.asarray(inp["att2_b1"], f).reshape(1, 256).astype(bf)
    d["a2w2"] = np.asarray(inp["att2_W2"], f).T.astype(bf)
    d["a2b2r"] = np.asarray(inp["att2_b2"], f).reshape(1, 256).astype(bf)
    d["g1a"] = (0.5 * np.asarray(inp["g1_W1"], f)[:, :768].T).astype(bf)
    d["g2a"] = (0.5 * np.asarray(inp["g2_W1"], f)[:, :768].T).astype(bf)
    # mem stored doubled (M=2mem) -> fold 0.5 into mem-part weights
    d["g1b"] = (0.5 * np.asarray(inp["g1_W1"], f)[:, 768:].T).astype(bf)
    d["g2b"] = (0.5 * np.asarray(inp["g2_W1"], f)[:, 768:].T).astype(bf)
    d["g1b1r"] = np.asarray(inp["g1_b1"], f).reshape(1, 256).astype(bf)
    d["g2b1r"] = np.asarray(inp["g2_b1"], f).reshape(1, 256).astype(bf)
    d["g1w2"] = np.asarray(inp["g1_W2"], f).T.astype(bf)
    d["g2w2"] = np.asarray(inp["g2_W2"], f).T.astype(bf)
    d["gb2c"] = (0.5 * np.concatenate(
        [np.asarray(inp["g1_b2"], f), np.asarray(inp["g2_b2"], f)]).reshape(4, 128).T).copy()
    # out MLP consumes doubled h and doubled mem -> fold 0.5 everywhere
    d["ow1"] = (0.5 * np.asarray(inp["out_W1"], f).T).astype(bf)
    d["ob1"] = np.asarray(inp["out_b1"], f).reshape(2, 128).T.copy()
    d["ow2"] = np.asarray(inp["out_W2"], f).T.astype(bf)
    d["ob2"] = np.asarray(inp["out_b2"], f).reshape(1, 1).copy()
    return d


def pack_x(x, core, Tp):
    xc = np.asarray(x[:, core * B:(core + 1) * B, :], np.float32)
    xt = xc.transpose(2, 0, 1).reshape(DIN, Tp * B)
    return np.concatenate([xt, np.ones((1, Tp * B), np.float32)], 0)


_CACHE = {}


def _get_program(Tp):
    if Tp not in _CACHE:
        _CACHE[Tp] = build_program(Tp)
    return _CACHE[Tp]


def kernel(**inputs):
    x = np.asarray(inputs["x"])
    Tp = x.shape[0]
    nc = _get_program(Tp)
    shared = pack_shared({k: np.asarray(v) for k, v in inputs.items()})
    in_maps = []
    for c in range(NCORES):
        m = dict(shared)
        m["xT"] = np.ascontiguousarray(pack_x(x, c, Tp))
        in_maps.append(m)
    res = run_bass_kernel_spmd(nc, in_maps, list(range(NCORES))).results
    out = np.concatenate([r["out"] for r in res], axis=0)
    return out.astype(np.float32)


if __name__ == "__main__":
    import time
    t0 = time.time()
    nc = build_program(32)
    print("built in", time.time() - t0, "s")
